# revision 61
# baseline (speedup 1.0000x reference)
"""F0Encoder Trainium2 kernel: 3x(conv1d+BN+relu+InterpLnr) + biLSTM, 8-core data parallel.

Strategy (v2):
- data parallel: 2 samples per core; BN batch stats via tiny AllReduce per (layer, mh)
- conv1d as K-chunked bf16 matmuls, (s,lt)-outer so psum banks retire early;
  per-bank stats ride the drains (scalar accum_out for sum, vector/gpsimd stt for sumsq)
  so the AllReduce launches ~1.5us after the conv ends
- AR(mh0) + BN(mh0 on vector) + transpose(mh0) all hide under conv(mh1);
  AR(mh1) hides under interp(mh0); BN(mh1) on scalar at half-L granularity
  pipelines with the transposes so interp(mh1) starts ASAP
- warmup AllReduce at t=0 absorbs CC init/barrier cost
- InterpLnr as block-banded bf16 matmuls (unchanged math), 4-pt-wide psum tiles
- LSTM: TC=16 chunks + BURN=16 burn-in -> 32 serial steps, 512 parallel
  sequences as 2 groups x 256 columns; xg staged via contiguous matmuls into
  xg_full then permuted into step-major layout with strided DVE copies;
  tanh(g) computed as 2*sigmoid(2g)-1 (g-gate weights pre-scaled 2x) so the
  whole gate block is one sigmoid ACT per group-step
"""

import numpy as np

import concourse.bass as bass
import concourse.mybir as mybir
import concourse.tile as tile
from concourse.tile import add_dep_helper
import bass_rust
from concourse.bass_utils import run_bass_kernel_spmd

dt = mybir.dt
AF = mybir.ActivationFunctionType
ALU = mybir.AluOpType
bf16 = np.float16

B, L, DF0, DE, H = 16, 2048, 257, 256, 32
MIN_SEG, MAX_SEG = 19, 32
MNS = L // MIN_SEG + 1          # 108 segments per sample
L2 = MAX_SEG * 2                # 64
EPS = 1e-5

NCORES = 8
SPC = B // NCORES               # 2 samples per core
TC = 16                         # LSTM chunk body length
BURN = 12                       # burn-in steps
S = TC + BURN                   # 32 serial steps
NCH = L // TC                   # 128 chunks per (sample, dir)
NGRP = 2
CPG = NCH // NGRP               # 64 chunks per group per quadrant
NSEQ = 4 * CPG                  # 256 cols per group: (q = d*2+s) x chunk
SAMP_T = [BURN + 7, BURN + 15]  # sampled steps (outputs every 8)
NPT = L // 128                  # 16 position tiles

XPAD = L + 4                    # conv padded length
PAD = TC                        # seqs pad on both sides
SPAD = L + 2 * PAD              # 2080

_MAX_WAITS = 1


def _fix_excess_waits(nc, max_waits=_MAX_WAITS):
    """walrus codegen rejects >1 sem wait per instruction; split extras onto
    preceding same-engine NOPs."""
    ctr = 0
    for fn in nc.m.functions:
        for bb in fn.blocks:
            insts = bb.instructions
            i = 0
            while i < len(insts):
                inst = insts[i]
                si = getattr(inst, "sync_info", None)
                if si is not None and len(si.on_wait) > max_waits:
                    waits = list(si.on_wait)
                    inst.sync_info = mybir.SyncInfo(
                        on_wait=waits[-max_waits:], on_update=list(si.on_update)
                    )
                    extra = waits[:-max_waits]
                    pos = i
                    for j in range(0, len(extra), max_waits):
                        nop = mybir.InstNoOp(name=f"wsplit_{ctr}", engine=inst.engine)
                        ctr += 1
                        nop.sync_info = mybir.SyncInfo(
                            on_wait=extra[j:j + max_waits], on_update=[]
                        )
                        insts.insert(pos, nop)
                        pos += 1
                        i += 1
                i += 1
    return ctr


# ---------------------------------------------------------------- host precompute

def _interp_indices(scales, lens):
    """Replicate reference interp_lnr index math in fp32.
    scales, lens: (B*MNS,) -> s1 (B,L) int64, lam (B,L) f32, nvalid (B,)"""
    scales = scales.reshape(B, MNS).astype(np.float32)
    lens = lens.reshape(B, MNS).astype(np.int64)
    s1 = np.zeros((B, L), np.int64)
    lam = np.zeros((B, L), np.float32)
    nval = np.zeros(B, np.int64)
    idx = np.arange(L2, dtype=np.float32)
    for b in range(B):
        pos = 0
        off = 0
        for g in range(MNS):
            sc = scales[b, g]
            ln = int(lens[b, g])
            isc = idx / sc                      # f32 division, as reference
            ifl = np.floor(isc)
            lm = isc - ifl
            ifl_i = ifl.astype(np.int64)
            m = (ifl < np.float32(ln - 1)) & ((ifl + np.float32(off)) < np.float32(L - 1))
            k = int(m.sum())
            take = min(k, L - pos)
            if take > 0:
                s1[b, pos:pos + take] = ifl_i[m][:take] + off
                lam[b, pos:pos + take] = lm[m][:take]
            pos += take
            off += ln
            if pos >= L:
                break
        nval[b] = pos
    return s1, lam, nval


def _build_g_blocks(s1_all, lam_all, nval_all):
    """blocks[l][pt] = union list of j-blocks over the whole batch (same for all
    cores -> one SPMD program); gdata[(l,b,pt,jb)] = (128,128) f32 G^T block."""
    blocks = []
    gdata = {}
    for l in range(3):
        s1 = s1_all[l]; lam = lam_all[l]; nval = nval_all[l]
        per_tile = []
        for pt in range(NPT):
            jset = set()
            for b in range(B):
                lo = pt * 128
                hi = min(int(nval[b]), (pt + 1) * 128)
                if hi <= lo:
                    continue
                v1 = s1[b, lo:hi]
                jset.add(int(v1.min()) // 128)
                jset.add((int(v1.max()) + 1) // 128)
            if not jset:
                jset = {min(pt, NPT - 1)}
            jlo, jhi = min(jset), min(max(jset), NPT - 1)
            per_tile.append(list(range(jlo, jhi + 1)))
        blocks.append(per_tile)
        for b in range(B):
            for pt in range(NPT):
                lo = pt * 128
                hi = min(int(nval[b]), (pt + 1) * 128)
                for jb in per_tile[pt]:
                    gm = np.zeros((128, 128), np.float32)
                    if hi > lo:
                        p = np.arange(lo, hi)
                        v1 = s1[b, lo:hi]
                        w2 = lam[b, lo:hi]
                        w1 = np.float32(1.0) - w2
                        r1 = v1 - jb * 128
                        m1 = (r1 >= 0) & (r1 < 128)
                        np.add.at(gm, (r1[m1], p[m1] - lo), w1[m1])
                        r2 = v1 + 1 - jb * 128
                        m2 = (r2 >= 0) & (r2 < 128)
                        np.add.at(gm, (r2[m2], p[m2] - lo), w2[m2])
                    gdata[(l, b, pt, jb)] = gm
    return blocks, gdata


def _gate_perm():
    # torch gate order i,f,g,o -> ours i,f,o,g
    return np.concatenate([np.arange(0, 64), np.arange(96, 128), np.arange(64, 96)])


def _host_prepare(inputs):
    x = np.asarray(inputs["x"], np.float32)            # (B, L, DF0)
    scales_raw = np.asarray(inputs["scales_raw"], np.float32)
    len_seg = np.asarray(inputs["len_seg"])

    s1_all, lam_all, nval_all = [], [], []
    for l in range(3):
        s1, lam, nv = _interp_indices(scales_raw[l] + np.float32(0.5), len_seg[l])
        s1_all.append(s1); lam_all.append(lam); nval_all.append(nv)
    blocks, gdata = _build_g_blocks(s1_all, lam_all, nval_all)

    # conv weights: cw{l} flat (128 k, 2 mh x 10 kd x 128 m)
    conv_w = []
    for wname in ["w0", "w1", "w2"]:
        w = np.asarray(inputs[wname], np.float32)      # (256, Cin, 5)
        flat = np.zeros((128, 20 * 128), np.float32)
        for mh in range(2):
            for kc in range(2):
                for d in range(5):
                    kd = kc * 5 + d
                    blk = w[mh * 128:(mh + 1) * 128, kc * 128:(kc + 1) * 128, d].T
                    flat[:, (mh * 10 + kd) * 128:(mh * 10 + kd + 1) * 128] = blk
        conv_w.append(flat)
    w0 = np.asarray(inputs["w0"], np.float32)
    cw0x = np.zeros((5, 256), np.float32)
    for mh in range(2):
        cw0x[:, mh * 128:(mh + 1) * 128] = w0[mh * 128:(mh + 1) * 128, 256, :].T

    gam = np.zeros((128, 6), np.float32)
    bet = np.zeros((128, 6), np.float32)
    for l, (g, be) in enumerate([("g0", "be0"), ("g1", "be1"), ("g2", "be2")]):
        gv = np.asarray(inputs[g], np.float32)
        bv = np.asarray(inputs[be], np.float32)
        for mh in range(2):
            gam[:, l * 2 + mh] = gv[mh * 128:(mh + 1) * 128]
            bet[:, l * 2 + mh] = bv[mh * 128:(mh + 1) * 128]

    perm = _gate_perm()
    wih = np.zeros((128, 512), np.float32)   # col (d*2+kc)*128+m
    whh = np.zeros((128, 256), np.float32)   # col d*128+m; rows replicated
                                             # per 32-block (quadrant bases)
    for d, sfx in enumerate(["f", "b"]):
        wi = np.asarray(inputs[f"wih_{sfx}"], np.float32)[perm]   # (128, 256)
        wh = np.asarray(inputs[f"whh_{sfx}"], np.float32)[perm]   # (128, 32)
        # g-gate rows (96:128 after perm) pre-scaled 2x: tanh(g)=2*sig(2g)-1
        wi = wi.copy(); wh = wh.copy()
        wi[96:128] *= 2.0
        wh[96:128] *= 2.0
        for kc in range(2):
            wih[:, (d * 2 + kc) * 128:(d * 2 + kc + 1) * 128] = \
                wi[:, kc * 128:(kc + 1) * 128].T
        for q in range(4):
            whh[q * 32:(q + 1) * 32, d * 128:(d + 1) * 128] = wh.T
        bsum = (np.asarray(inputs[f"bih_{sfx}"], np.float32)
                + np.asarray(inputs[f"bhh_{sfx}"], np.float32))
        assert np.all(bsum == 0.0), "nonzero LSTM biases unsupported"

    xcm = np.transpose(x, (0, 2, 1))                    # (B, 257, L)
    nblk_layer = [sum(len(blocks[l][pt]) for pt in range(NPT)) for l in range(3)]
    in_maps = []
    for core in range(NCORES):
        sl = slice(core * SPC, (core + 1) * SPC)
        xp = np.zeros((SPC, DF0, XPAD), np.float32)
        xp[:, :, 2:2 + L] = xcm[sl]
        x5 = np.zeros((SPC, 5, XPAD), np.float32)
        ext = np.zeros((SPC, XPAD + 4), np.float32)
        ext[:, :XPAD] = xp[:, 256]
        for r in range(5):
            x5[:, r, :] = ext[:, r:r + XPAD]
        gl = []
        for l in range(3):
            for s in range(SPC):
                b = core * SPC + s
                for pt in range(NPT):
                    for jb in blocks[l][pt]:
                        gl.append(gdata[(l, b, pt, jb)])
        gblk = np.stack(gl)                              # (NBLK, 128, 128)
        gflat = gblk.transpose(1, 0, 2).reshape(128, -1)  # (128, NBLK*128)
        in_maps.append({
            "wz": np.zeros((128, 2), np.float32),
            "x": xp[:, :256].astype(bf16),
            "x5": x5.astype(bf16),
            "cw0": conv_w[0].astype(bf16), "cw0x": cw0x.astype(bf16),
            "cw1": conv_w[1].astype(bf16), "cw2": conv_w[2].astype(bf16),
            "gam": gam, "bet": bet,
            "gblk": gflat.astype(bf16),
            "wih": wih.astype(bf16), "whh": whh.astype(bf16),
            "ident": np.eye(128, dtype=bf16),
        })
    meta = {"blocks": blocks, "nblk_layer": nblk_layer,
            "nblk_total": sum(nblk_layer) * SPC}
    return in_maps, meta


# ---------------------------------------------------------------- device program

def _neg_ap(tile_ap, col0, step1, count1, step2, count2):
    """strided (possibly negative) 2-level free AP over a [128, N] tile."""
    ap = tile_ap.copy()
    p0 = list(ap.ap[0])
    ap.ap = bass_rust.VecI64Pair([p0, [step1, count1], [step2, count2]])
    ap.offset = ap.offset + col0
    return ap


def _build_program(meta, debug=False):
    blocks = meta["blocks"]
    nblk_layer = meta["nblk_layer"]

    nc = bass.Bass()
    if debug:
        dbg_seqs_d = nc.dram_tensor("dbg_seqs", [SPC, 2, 128, SPAD],
                                    dt.float16, kind="ExternalOutput")
        dbg_xg_d = nc.dram_tensor("dbg_xg", [128, 4 * SPAD],
                                  dt.float16, kind="ExternalOutput")
        dbg_y_d = nc.dram_tensor("dbg_y", [3, SPC, 2, 128, L], dt.float32,
                                 kind="ExternalOutput")
        dbg_int_d = nc.dram_tensor("dbg_int", [3, SPC, 2, 128, XPAD],
                                   dt.float16, kind="ExternalOutput")
        dbg_zt_d = nc.dram_tensor("dbg_zt", [3, SPC, 2, 128, NPT * 128],
                                  dt.float16, kind="ExternalOutput")
        dbg_gb_d = nc.dram_tensor("dbg_gb", [128, meta["nblk_total"] * 128],
                                  dt.float16, kind="ExternalOutput")
    wz_d = nc.dram_tensor("wz", [128, 2], dt.float32, kind="ExternalInput")
    x_d = nc.dram_tensor("x", [SPC, 256, XPAD], dt.float16, kind="ExternalInput")
    x5_d = nc.dram_tensor("x5", [SPC, 5, XPAD], dt.float16, kind="ExternalInput")
    cw_d = [nc.dram_tensor(f"cw{l}", [128, 20 * 128], dt.float16,
                           kind="ExternalInput") for l in range(3)]
    cw0x_d = nc.dram_tensor("cw0x", [5, 256], dt.float16, kind="ExternalInput")
    gam_d = nc.dram_tensor("gam", [128, 6], dt.float32, kind="ExternalInput")
    bet_d = nc.dram_tensor("bet", [128, 6], dt.float32, kind="ExternalInput")
    gblk_d = nc.dram_tensor("gblk", [128, meta["nblk_total"] * 128], dt.float16,
                            kind="ExternalInput")
    wih_d = nc.dram_tensor("wih", [128, 512], dt.float16, kind="ExternalInput")
    whh_d = nc.dram_tensor("whh", [128, 256], dt.float16, kind="ExternalInput")
    ident_d = nc.dram_tensor("ident", [128, 128], dt.float16, kind="ExternalInput")
    hout_d = nc.dram_tensor("hout", [128, 256], dt.float32,
                            kind="ExternalOutput")

    lay_off = [0, SPC * nblk_layer[0], SPC * (nblk_layer[0] + nblk_layer[1])]
    inv_n = 1.0 / (B * L)
    groups = [list(range(NCORES))]

    with tile.TileContext(nc) as tc:
        with (
            tc.tile_pool(name="const", bufs=1) as cp,
            tc.tile_pool(name="bufs", bufs=1) as bp,
            tc.tile_pool(name="dram", bufs=2, space="DRAM") as dp,
        ):
            # ---- warmup AllReduce: reads host-provided zeros directly from
            # DRAM (no on-chip deps), first on the gpsimd queue so the CC
            # trigger fires at engine start and the mesh absorbs core-start
            # skew while conv runs
            wsin = dp.tile([128, 2], dt.float32, tag="wcin", name="wcin")
            wsout = dp.tile([128, 2], dt.float32, tag="wcout", name="wcout")
            nc.sync.dma_start(wsin[:], wz_d[:])
            nc.gpsimd.collective_compute(
                "AllReduce", ALU.add, replica_groups=[list(range(NCORES))],
                ins=[wsin.opt()], outs=[wsout.opt()])
            # ---- constants: critical loads (x, cw0, x5) on scalar queue
            # first; everything else deferred onto the gpsimd queue
            xa = [[bp.tile([128, XPAD], dt.float16, tag=f"xa{s}{h}",
                           name=f"xa{s}{h}")
                   for h in range(2)] for s in range(SPC)]
            xb = [[bp.tile([128, XPAD], dt.float16, tag=f"xb{s}{h}",
                           name=f"xb{s}{h}")
                   for h in range(2)] for s in range(SPC)]
            x5t = [bp.tile([5, XPAD], dt.float16, tag=f"x5{s}", name=f"x5t{s}")
                   for s in range(SPC)]
            seqs = [[bp.tile([128, SPAD], dt.float16, tag=f"sq{s}{h}",
                             name=f"sq{s}{h}")
                     for h in range(2)] for s in range(SPC)]
            cw = [cp.tile([128, 20 * 128], dt.float16, tag=f"cw{l}",
                          name=f"cw{l}")
                  for l in range(3)]
            cw0x = cp.tile([5, 256], dt.float16)
            for h in range(2):
                nc.scalar.dma_start(xa[0][h][:], x_d[0, h * 128:(h + 1) * 128, :])
            nc.scalar.dma_start(cw[0][:], cw_d[0][:])
            nc.scalar.dma_start(cw0x[:], cw0x_d[:])
            nc.scalar.dma_start(x5t[0][:], x5_d[0])
            for h in range(2):
                nc.scalar.dma_start(xa[1][h][:], x_d[1, h * 128:(h + 1) * 128, :])
            nc.scalar.dma_start(x5t[1][:], x5_d[1])
            # preload ACT sigmoid/tanh tables during the idle startup so the
            # first LSTM activation doesn't eat the table-load latency
            warm = cp.tile([1, 2], dt.float32)
            nc.vector.memset(warm[:], 0.0)
            nc.scalar.activation(warm[:, 0:1], warm[:, 0:1], AF.Sigmoid)
            nc.scalar.activation(warm[:, 1:2], warm[:, 1:2], AF.Tanh)
            gam = cp.tile([128, 6], dt.float32)
            bet = cp.tile([128, 6], dt.float32)
            nc.gpsimd.dma_start(gam[:], gam_d[:])
            nc.gpsimd.dma_start(bet[:], bet_d[:])
            wih = cp.tile([128, 512], dt.float16)
            nc.gpsimd.dma_start(wih[:], wih_d[:])
            whh = cp.tile([128, 256], dt.float16)
            nc.gpsimd.dma_start(whh[:], whh_d[:])
            ident = cp.tile([128, 128], dt.float16)
            nc.gpsimd.dma_start(ident[:], ident_d[:])
            nc.gpsimd.dma_start(cw[1][:], cw_d[1][:])
            nc.gpsimd.dma_start(cw[2][:], cw_d[2][:])
            for s in range(SPC):
                for h in range(2):
                    nc.vector.memset(xb[s][h][:, 0:2], 0.0)
                    nc.vector.memset(xb[s][h][:, XPAD - 2:XPAD], 0.0)
                    nc.vector.memset(seqs[s][h][:, 0:PAD], 0.0)
                    nc.vector.memset(seqs[s][h][:, SPAD - PAD:SPAD], 0.0)

            # ================================ conv + interp layers
            with (
                tc.tile_pool(name="convbuf", bufs=1) as cvp,
                tc.tile_pool(name="scratch", bufs=2) as scr,
                tc.tile_pool(name="bnscr", bufs=1) as bns,
                tc.tile_pool(name="cpsum", bufs=4, space="PSUM") as cps,
                tc.tile_pool(name="ipsum", bufs=2, space="PSUM") as ipp,
                tc.tile_pool(name="tpsum", bufs=2, space="PSUM") as tpp,
            ):
                y = [[cvp.tile([128, L], dt.float32, tag=f"y{s}{h}",
                               name=f"y{s}{h}")
                      for h in range(2)] for s in range(SPC)]
                zt = [[cvp.tile([128, NPT * 128], dt.float16, tag=f"zt{s}{h}",
                                name=f"zt{s}{h}")
                       for h in range(2)] for s in range(SPC)]
                gbuf = cvp.tile([128, meta["nblk_total"] * 128], dt.float16,
                                tag="gb")
                # layer-0 G blocks load now; l1/l2 are deferred until after
                # the first stats AllReduce is triggered so its tiny input
                # DMA isn't queued behind megabytes of G-block descriptors
                a1_l0 = (lay_off[0] + SPC * nblk_layer[0]) * 128
                nc.gpsimd.dma_start(gbuf[:, 0:a1_l0], gblk_d[:, 0:a1_l0])
                sacc = cvp.tile([128, 16], dt.float32)
                qacc = cvp.tile([128, 16], dt.float32)
                stats = cvp.tile([128, 4], dt.float32)
                statsg = cvp.tile([128, 4], dt.float32)
                abt = cvp.tile([128, 4], dt.float32)
                t2 = cvp.tile([128, 2], dt.float32)
                epst = cvp.tile([128, 1], dt.float32)
                nc.vector.memset(epst[:], EPS)
                bnt = bns.tile([128, L // 2], dt.float32, tag="bnt")

                cur, nxt = xa, xb
                for l in range(3):
                    nkd = 11 if l == 0 else 10
                    per_pt_off = {}
                    off = 0
                    for pt in range(NPT):
                        per_pt_off[pt] = off
                        off += len(blocks[l][pt])

                    souts = []

                    def conv_bank(mh, s, lt):
                        ps = cps.tile([128, 512], dt.float32, tag="cps")
                        for kd in range(nkd):
                            if kd < 10:
                                lhs = cw[l][:, (mh * 10 + kd) * 128:
                                            (mh * 10 + kd + 1) * 128]
                                kc, d = divmod(kd, 5)
                                rhs = cur[s][kc][:, lt * 512 + d:
                                                 lt * 512 + d + 512]
                            else:
                                lhs = cw0x[:, mh * 128:(mh + 1) * 128]
                                rhs = x5t[s][:, lt * 512:lt * 512 + 512]
                            nc.tensor.matmul(ps[:], lhs, rhs,
                                             start=(kd == 0),
                                             stop=(kd == nkd - 1))
                        k = mh * 8 + s * 4 + lt
                        ysl = y[s][mh][:, lt * 512:(lt + 1) * 512]
                        nc.scalar.activation(ysl, ps[:], AF.Copy,
                                             accum_out=sacc[:, k:k + 1])
                        sq = scr.tile([128, 512], dt.float32, tag="sq")
                        nc.scalar.activation(sq[:], ps[:], AF.Square,
                                             accum_out=qacc[:, k:k + 1])

                    def emit_stats(mhs):
                        eng = nc.vector
                        for mh in mhs:
                            eng.tensor_reduce(
                                stats[:, 2 * mh:2 * mh + 1],
                                sacc[:, mh * 8:mh * 8 + 8],
                                mybir.AxisListType.X, ALU.add)
                            eng.tensor_reduce(
                                stats[:, 2 * mh + 1:2 * mh + 2],
                                qacc[:, mh * 8:mh * 8 + 8],
                                mybir.AxisListType.X, ALU.add)
                        w = 2 * len(mhs)
                        mh0 = mhs[0]
                        sin = dp.tile([128, w], dt.float32, tag=f"cin{w}",
                                      name=f"cin{l}{mh0}")
                        sout = dp.tile([128, w], dt.float32, tag=f"cout{w}",
                                       name=f"cout{l}{mh0}")
                        nc.scalar.dma_start(sin[:],
                                            stats[:, 2 * mh0:2 * mh0 + w])
                        nc.gpsimd.collective_compute(
                            "AllReduce", ALU.add, replica_groups=groups,
                            ins=[sin.opt()], outs=[sout.opt()])
                        souts.append(sout)
                        nc.sync.dma_start(statsg[:, 2 * mh0:2 * mh0 + w],
                                          sout[:])

                    def coef_pre(mh):
                        sm = statsg[:, 2 * mh:2 * mh + 1]
                        qm = statsg[:, 2 * mh + 1:2 * mh + 2]
                        nc.vector.scalar_tensor_tensor(
                            t2[:, mh:mh + 1], sm, inv_n, sm, ALU.mult, ALU.mult)
                        nc.vector.tensor_tensor(t2[:, mh:mh + 1], qm,
                                                t2[:, mh:mh + 1], ALU.subtract)

                    def coef_sqrt(mh):
                        nc.scalar.activation(t2[:, mh:mh + 1], t2[:, mh:mh + 1],
                                             AF.Sqrt, bias=epst[:], scale=inv_n)

                    def coef_post(mh):
                        sm = statsg[:, 2 * mh:2 * mh + 1]
                        nc.vector.reciprocal(t2[:, mh:mh + 1], t2[:, mh:mh + 1])
                        nc.vector.tensor_tensor(
                            abt[:, mh:mh + 1], gam[:, 2 * l + mh:2 * l + mh + 1],
                            t2[:, mh:mh + 1], ALU.mult)
                        nc.vector.scalar_tensor_tensor(
                            t2[:, mh:mh + 1], sm, inv_n, abt[:, mh:mh + 1],
                            ALU.mult, ALU.mult)
                        nc.vector.tensor_tensor(
                            abt[:, 2 + mh:3 + mh],
                            bet[:, 2 * l + mh:2 * l + mh + 1],
                            t2[:, mh:mh + 1], ALU.subtract)

                    def emit_interp(mh):
                        deng = nc.vector if mh == 0 else nc.scalar
                        for s in range(SPC):
                            sbase = lay_off[l] + s * nblk_layer[l]
                            for w in range(4):
                                pts = list(range(4 * w, 4 * w + 4))
                                psw = ipp.tile([128, 512], dt.float32,
                                               tag="ipw", name=f"ipw{w}")
                                for pt in pts:
                                    bl = blocks[l][pt]
                                    k = pt - 4 * w
                                    for jb in bl:
                                        lhs = zt[s][mh][:, jb * 128:
                                                        (jb + 1) * 128]
                                        gi = sbase + per_pt_off[pt] + bl.index(jb)
                                        rhs = gbuf[:, gi * 128:(gi + 1) * 128]
                                        nc.tensor.matmul(
                                            psw[:, k * 128:(k + 1) * 128],
                                            lhs, rhs,
                                            start=(jb == bl[0]),
                                            stop=(jb == bl[-1]))
                                if l < 2:
                                    dst = nxt[s][mh][:, 2 + 512 * w:
                                                     2 + 512 * (w + 1)]
                                else:
                                    dst = seqs[s][mh][:, PAD + 512 * w:
                                                      PAD + 512 * (w + 1)]
                                if mh == 0:
                                    deng.tensor_copy(dst, psw[:])
                                else:
                                    deng.copy(dst, psw[:])

                    def emit_transpose(mh):
                        # PE-transpose BN'd z [128ch, L] into position-major
                        # zt tiles; 4 tiles share one psum bank, one drain
                        deng = nc.vector if mh == 0 else nc.scalar
                        for s in range(SPC):
                            src = nxt[s][mh]
                            for w in range(4):
                                tp = tpp.tile([128, 512], dt.float16,
                                              tag="tp", name=f"tp{w}")
                                for k in range(4):
                                    pt = 4 * w + k
                                    nc.tensor.transpose(
                                        tp[:, k * 128:(k + 1) * 128],
                                        src[:, 2 + 128 * pt:2 + 128 * (pt + 1)],
                                        ident[:])
                                dst = zt[s][mh][:, 512 * w:512 * (w + 1)]
                                if mh == 0:
                                    deng.tensor_copy(dst, tp[:])
                                else:
                                    deng.copy(dst, tp[:])

                    # ---- conv mh0 (+ AR0 mid-conv for l>0; layer 0's first
                    # AR is core-skew-bound anyway, so it carries BOTH
                    # halves' stats after conv ends -> one mesh, not two)
                    for s in range(SPC):
                        for lt in range(4):
                            conv_bank(0, s, lt)
                    if l > 0:
                        emit_stats([0])
                        coef_pre(0)
                    # ---- conv mh1; sqrt0 interleaves into the scalar stream
                    # after bank 2 so it runs mid-conv-mh1 once AR0 lands
                    banks1 = [(s, lt) for s in range(SPC) for lt in range(4)]
                    for bi, (s, lt) in enumerate(banks1):
                        if bi == 3 and l > 0:
                            coef_sqrt(0)
                        conv_bank(1, s, lt)
                    if l == 0:
                        emit_stats([0, 1])
                        # deferred G-block loads: enqueue behind the AR
                        # trigger so the stats DMA isn't ring-blocked
                        a1_l2 = (lay_off[2] + SPC * nblk_layer[2]) * 128
                        nc.gpsimd.dma_start(gbuf[:, a1_l0:a1_l2],
                                            gblk_d[:, a1_l0:a1_l2])
                        coef_pre(0)
                        coef_sqrt(0)
                    coef_post(0)
                    # ---- BN0 split across DVE (s0) + ACT (s1) so the
                    # post-AR tail is short
                    for hf in range(2):
                        ysl = y[0][0][:, hf * 1024:(hf + 1) * 1024]
                        ztar = nxt[0][0][:, 2 + hf * 1024:2 + (hf + 1) * 1024]
                        nc.vector.tensor_scalar_mul(bnt[:], ysl, abt[:, 0:1])
                        nc.vector.tensor_scalar(ztar, bnt[:], abt[:, 2:3],
                                                0.0, ALU.add, ALU.max)
                    for hf in range(2):
                        ysl = y[1][0][:, hf * 1024:(hf + 1) * 1024]
                        ztar = nxt[1][0][:, 2 + hf * 1024:2 + (hf + 1) * 1024]
                        nc.scalar.activation(ztar, ysl, AF.Relu,
                                             bias=abt[:, 2:3],
                                             scale=abt[:, 0:1])
                    emit_transpose(0)
                    if l > 0:
                        emit_stats([1])
                    # ---- interp mh0 (hides AR1)
                    emit_interp(0)
                    # ---- coefs + BN1 on scalar at half-L granularity
                    coef_pre(1)
                    coef_sqrt(1)
                    coef_post(1)
                    for hf in range(2):
                        ysl = y[0][1][:, hf * 1024:(hf + 1) * 1024]
                        ztar = nxt[0][1][:, 2 + hf * 1024:2 + (hf + 1) * 1024]
                        nc.scalar.activation(ztar, ysl, AF.Relu,
                                             bias=abt[:, 3:4],
                                             scale=abt[:, 1:2])
                    for hf in range(2):
                        ysl = y[1][1][:, hf * 1024:(hf + 1) * 1024]
                        ztar = nxt[1][1][:, 2 + hf * 1024:2 + (hf + 1) * 1024]
                        nc.vector.tensor_scalar_mul(bnt[:], ysl, abt[:, 1:2])
                        nc.vector.tensor_scalar(ztar, bnt[:], abt[:, 3:4],
                                                0.0, ALU.add, ALU.max)
                    emit_transpose(1)
                    emit_interp(1)
                    if debug:
                        for s in range(SPC):
                            for h in range(2):
                                nc.sync.dma_start(dbg_y_d[l, s, h], y[s][h][:])
                                nc.sync.dma_start(dbg_zt_d[l, s, h],
                                                  zt[s][h][:])
                                if l < 2:
                                    nc.sync.dma_start(dbg_int_d[l, s, h],
                                                      nxt[s][h][:])
                        if l == 0:
                            nc.sync.dma_start(dbg_gb_d[:], gbuf[:])
                    if l < 2:
                        cur, nxt = nxt, cur

            # ================================ LSTM via Picard iteration
            # Quadrant q = d*2+s (d=dir, s=sample). Per iteration:
            #   G_q = Wih_d x_q (+ Whh_d h_prev_q shifted by 1) via matmuls
            #   sg = sigmoid(G) over all 128 gate rows (g rows pre-scaled
            #        2x on host: tanh(g) = 2 sig(2g) - 1)
            #   u  = 2 sg_i sg_g - sg_i               (DVE)
            #   c  = tensor_tensor_scan(sg_f, u)      (c_t = f c_{t-1} + u_t)
            #   h  = sg_o tanh(c)                     (ACT + DVE)
            # bwd quadrants read seqs reversed (manual AP) and are emitted
            # after all fwd matmuls so tensor program order covers the
            # untracked reads. h lives at col t+1 (col 0 = zeros) so the
            # Whh matmul for chunk c reads h_{t-1} as cols [c*512, c*512+512).
            NIT = 3
            CH = 512
            NCHK = L // CH
            with (
                tc.tile_pool(name="lstm", bufs=1) as lp,
                tc.tile_pool(name="work", bufs=2) as wp,
                tc.tile_pool(name="psg", bufs=4, space="PSUM") as gp,
                tc.tile_pool(name="pst", bufs=1, space="PSUM") as tp2,
            ):
                sgt = lp.tile([128, 4 * L], dt.float16, tag="sgt", name="sgt")
                # xg cached in fp16: computed by matmul once (it=0), replayed
                # into psum via identity matmul for later iterations' whh
                # accumulation (cheaper than recomputing the 2-block xg)
                xgs = lp.tile([128, 4 * L], dt.float16, tag="xgs", name="xgs")
                # quadrant-packed [128 = 4q x 32, L] gate planes: DVE ops are
                # column-bound regardless of rows, so every elementwise op
                # runs once on all 4 quadrants. DMA does the partition-
                # crossing repack (rings are idle here).
                gpk = [lp.tile([128, L], dt.float16, tag=f"gp{g}",
                               name=f"gpk{g}") for g in range(4)]  # i,f,o,g
                t1 = lp.tile([128, L], dt.float16, tag="t1", name="t1")
                upk = lp.tile([128, L], dt.float16, tag="upk", name="upk")
                # h_t at col t+1 (col 0 zero); whh is row-replicated so its
                # lhsT base matches the packed rhs base per quadrant
                hpk = lp.tile([128, L + 1], dt.float16, tag="hpk", name="hpk")
                # PE base partitions are limited to 0/32/64: quadrant 3
                # (base 96) gets a DMA-unpacked copy at base 0
                hq3 = lp.tile([32, L + 1], dt.float16, tag="hq3", name="hq3")
                cbuf = lp.tile([128, L], dt.float16, tag="cbuf", name="cbuf")
                # tanh(c) lands in PSUM so the h multiply (opk SBUF + tcb
                # PSUM) dodges the equal-base-partition rule
                tcb = tp2.tile([128, L], dt.float32, tag="tcb", name="tcb")
                hfin = lp.tile([128, 256], dt.float32, tag="hfin",
                               name="hfin")
                nc.vector.memset(hpk[:, 0:1], 0.0)

                for it in range(NIT):
                    for d in range(2):
                        for s in range(SPC):
                            q = d * 2 + s
                            for c in range(NCHK):
                                xsl = xgs[:, q * L + c * CH:
                                          q * L + (c + 1) * CH]
                                ps = gp.tile([128, CH], dt.float32, tag="pg",
                                             name=f"pg{q}{c}")
                                if it == 0:
                                    for half in range(CH // 512):
                                        c0 = c * CH + half * 512
                                        pssl = ps[:, half * 512:
                                                  (half + 1) * 512]
                                        for kc in range(2):
                                            lhs = wih[:, (d * 2 + kc) * 128:
                                                      (d * 2 + kc + 1) * 128]
                                            if d == 0:
                                                rhs = seqs[s][kc][
                                                    :, PAD + c0:
                                                    PAD + c0 + 512]
                                            else:
                                                rhs = _neg_ap(
                                                    seqs[s][kc][:],
                                                    PAD + (L - 1) - c0,
                                                    -1, 512, 0, 1)
                                            nc.tensor.matmul(
                                                pssl, lhs, rhs,
                                                start=(kc == 0),
                                                stop=(kc == 1))
                                    nc.vector.tensor_copy(xsl, ps[:])
                                else:
                                    if q < 3:
                                        wsl = whh[q * 32:(q + 1) * 32,
                                                  d * 128:(d + 1) * 128]
                                    else:
                                        wsl = whh[0:32,
                                                  d * 128:(d + 1) * 128]
                                    for half in range(CH // 512):
                                        c0 = c * CH + half * 512
                                        pssl = ps[:, half * 512:
                                                  (half + 1) * 512]
                                        nc.tensor.matmul(
                                            pssl, ident[:],
                                            xgs[:, q * L + c0:
                                                q * L + c0 + 512],
                                            start=True, stop=False)
                                        if q < 3:
                                            hsl = hpk[q * 32:(q + 1) * 32,
                                                      c0:c0 + 512]
                                        else:
                                            hsl = hq3[:, c0:c0 + 512]
                                        nc.tensor.matmul(pssl, wsl, hsl,
                                                         start=False,
                                                         stop=True)
                                nc.scalar.activation(
                                    sgt[:, q * L + c * CH:
                                        q * L + (c + 1) * CH],
                                    ps[:], AF.Sigmoid)
                            # repack this quadrant's gate planes right away
                            # (overlaps the next quadrant's matmuls)
                            qc = slice(q * L, (q + 1) * L)
                            for g, eng in ((3, nc.sync), (0, nc.gpsimd),
                                           (1, nc.sync), (2, nc.gpsimd)):
                                eng.dma_start(gpk[g][q * 32:(q + 1) * 32, :],
                                              sgt[g * 32:(g + 1) * 32, qc])
                    # u = sig(i) * (2*sig(2g) - 1), c = scan: f*c + u,
                    # h = sig(o) tanh(c); everything chunked at 512 so the
                    # chain pipelines and the first h chunk (which unblocks
                    # the next iteration's whh matmuls) lands early
                    for c in range(L // 512):
                        cs = slice(c * 512, (c + 1) * 512)
                        nc.vector.tensor_scalar(t1[:, cs], gpk[3][:, cs],
                                                2.0, 1.0, ALU.mult,
                                                ALU.subtract)
                        nc.vector.tensor_tensor(upk[:, cs], t1[:, cs],
                                                gpk[0][:, cs], ALU.mult)
                        init = 0.0 if c == 0 else cbuf[:, c * 512 - 1:c * 512]
                        nc.vector.tensor_tensor_scan(
                            cbuf[:, cs], gpk[1][:, cs], upk[:, cs],
                            init, ALU.mult, ALU.add)
                        nc.scalar.activation(tcb[:, cs], cbuf[:, cs], AF.Tanh)
                        nc.vector.tensor_tensor(
                            hpk[:, 1 + c * 512:1 + (c + 1) * 512],
                            gpk[2][:, cs], tcb[:, cs], ALU.mult)
                        if it < NIT - 1:
                            nc.gpsimd.dma_start(
                                hq3[:, 1 + c * 512:1 + (c + 1) * 512],
                                hpk[96:128, 1 + c * 512:1 + (c + 1) * 512])

                # fwd needs h at t=8j+7 (col 8j+8 of hpk); bwd (stored
                # reversed) needs h_rev[L-1-8j] (col L-8j).
                for q in range(4):
                    src = hpk[q * 32:(q + 1) * 32, :].copy()
                    p0 = list(src.ap[0])
                    if q < 2:
                        src.ap = bass_rust.VecI64Pair([p0, [8, 256]])
                        src.offset = src.offset + 8
                    else:
                        src.ap = bass_rust.VecI64Pair([p0, [-8, 256]])
                        src.offset = src.offset + L
                    nc.vector.tensor_copy(hfin[q * 32:(q + 1) * 32, :], src)
                nc.sync.dma_start(hout_d[:], hfin[:])
                if debug:
                    for s in range(SPC):
                        for h in range(2):
                            nc.sync.dma_start(dbg_seqs_d[s, h], seqs[s][h][:])

    return nc


# ---------------------------------------------------------------- entry point

def _gather(res):
    """hout (128, 256) per core (rows = (d*2+s)*32 + hdim, cols = output
    position j) -> full (B, 256, 64) output."""
    out = np.zeros((B, 256, 64), np.float32)
    for core in range(NCORES):
        ho = res.results[core]["hout"]          # (128, 256)
        for s in range(SPC):
            bidx = core * SPC + s
            out[bidx, :, 0:32] = ho[s * 32:(s + 1) * 32, :].T
            out[bidx, :, 32:64] = ho[(2 + s) * 32:(3 + s) * 32, :].T
    return out


def kernel(**inputs):
    in_maps, meta = _host_prepare(inputs)
    nc = _build_program(meta)
    _fix_excess_waits(nc)
    res = run_bass_kernel_spmd(nc, in_maps, list(range(NCORES)))
    return _gather(res)



# revision 62
# speedup vs baseline: 1.0965x; 1.0965x over previous
"""F0Encoder Trainium2 kernel: 3x(conv1d+BN+relu+InterpLnr) + biLSTM, 8-core data parallel.

Strategy (v2):
- data parallel: 2 samples per core; BN batch stats via tiny AllReduce per (layer, mh)
- conv1d as K-chunked bf16 matmuls, (s,lt)-outer so psum banks retire early;
  per-bank stats ride the drains (scalar accum_out for sum, vector/gpsimd stt for sumsq)
  so the AllReduce launches ~1.5us after the conv ends
- AR(mh0) + BN(mh0 on vector) + transpose(mh0) all hide under conv(mh1);
  AR(mh1) hides under interp(mh0); BN(mh1) on scalar at half-L granularity
  pipelines with the transposes so interp(mh1) starts ASAP
- warmup AllReduce at t=0 absorbs CC init/barrier cost
- InterpLnr as block-banded bf16 matmuls (unchanged math), 4-pt-wide psum tiles
- LSTM: TC=16 chunks + BURN=16 burn-in -> 32 serial steps, 512 parallel
  sequences as 2 groups x 256 columns; xg staged via contiguous matmuls into
  xg_full then permuted into step-major layout with strided DVE copies;
  tanh(g) computed as 2*sigmoid(2g)-1 (g-gate weights pre-scaled 2x) so the
  whole gate block is one sigmoid ACT per group-step
"""

import numpy as np

import concourse.bass as bass
import concourse.mybir as mybir
import concourse.tile as tile
from concourse.tile import add_dep_helper
import bass_rust
from concourse.bass_utils import run_bass_kernel_spmd

dt = mybir.dt
AF = mybir.ActivationFunctionType
ALU = mybir.AluOpType
bf16 = np.float16

B, L, DF0, DE, H = 16, 2048, 257, 256, 32
MIN_SEG, MAX_SEG = 19, 32
MNS = L // MIN_SEG + 1          # 108 segments per sample
L2 = MAX_SEG * 2                # 64
EPS = 1e-5

NCORES = 8
SPC = B // NCORES               # 2 samples per core
TC = 16                         # LSTM chunk body length
BURN = 12                       # burn-in steps
S = TC + BURN                   # 32 serial steps
NCH = L // TC                   # 128 chunks per (sample, dir)
NGRP = 2
CPG = NCH // NGRP               # 64 chunks per group per quadrant
NSEQ = 4 * CPG                  # 256 cols per group: (q = d*2+s) x chunk
SAMP_T = [BURN + 7, BURN + 15]  # sampled steps (outputs every 8)
NPT = L // 128                  # 16 position tiles

XPAD = L + 4                    # conv padded length
PAD = TC                        # seqs pad on both sides
SPAD = L + 2 * PAD              # 2080

_MAX_WAITS = 1


def _fix_excess_waits(nc, max_waits=_MAX_WAITS):
    """walrus codegen rejects >1 sem wait per instruction; split extras onto
    preceding same-engine NOPs."""
    ctr = 0
    for fn in nc.m.functions:
        for bb in fn.blocks:
            insts = bb.instructions
            i = 0
            while i < len(insts):
                inst = insts[i]
                si = getattr(inst, "sync_info", None)
                if si is not None and len(si.on_wait) > max_waits:
                    waits = list(si.on_wait)
                    inst.sync_info = mybir.SyncInfo(
                        on_wait=waits[-max_waits:], on_update=list(si.on_update)
                    )
                    extra = waits[:-max_waits]
                    pos = i
                    for j in range(0, len(extra), max_waits):
                        nop = mybir.InstNoOp(name=f"wsplit_{ctr}", engine=inst.engine)
                        ctr += 1
                        nop.sync_info = mybir.SyncInfo(
                            on_wait=extra[j:j + max_waits], on_update=[]
                        )
                        insts.insert(pos, nop)
                        pos += 1
                        i += 1
                i += 1
    return ctr


# ---------------------------------------------------------------- host precompute

def _interp_indices(scales, lens):
    """Replicate reference interp_lnr index math in fp32.
    scales, lens: (B*MNS,) -> s1 (B,L) int64, lam (B,L) f32, nvalid (B,)"""
    scales = scales.reshape(B, MNS).astype(np.float32)
    lens = lens.reshape(B, MNS).astype(np.int64)
    s1 = np.zeros((B, L), np.int64)
    lam = np.zeros((B, L), np.float32)
    nval = np.zeros(B, np.int64)
    idx = np.arange(L2, dtype=np.float32)
    for b in range(B):
        pos = 0
        off = 0
        for g in range(MNS):
            sc = scales[b, g]
            ln = int(lens[b, g])
            isc = idx / sc                      # f32 division, as reference
            ifl = np.floor(isc)
            lm = isc - ifl
            ifl_i = ifl.astype(np.int64)
            m = (ifl < np.float32(ln - 1)) & ((ifl + np.float32(off)) < np.float32(L - 1))
            k = int(m.sum())
            take = min(k, L - pos)
            if take > 0:
                s1[b, pos:pos + take] = ifl_i[m][:take] + off
                lam[b, pos:pos + take] = lm[m][:take]
            pos += take
            off += ln
            if pos >= L:
                break
        nval[b] = pos
    return s1, lam, nval


def _build_g_blocks(s1_all, lam_all, nval_all):
    """blocks[l][pt] = union list of j-blocks over the whole batch (same for all
    cores -> one SPMD program); gdata[(l,b,pt,jb)] = (128,128) f32 G^T block."""
    blocks = []
    gdata = {}
    for l in range(3):
        s1 = s1_all[l]; lam = lam_all[l]; nval = nval_all[l]
        per_tile = []
        for pt in range(NPT):
            jset = set()
            for b in range(B):
                lo = pt * 128
                hi = min(int(nval[b]), (pt + 1) * 128)
                if hi <= lo:
                    continue
                v1 = s1[b, lo:hi]
                jset.add(int(v1.min()) // 128)
                jset.add((int(v1.max()) + 1) // 128)
            if not jset:
                jset = {min(pt, NPT - 1)}
            jlo, jhi = min(jset), min(max(jset), NPT - 1)
            per_tile.append(list(range(jlo, jhi + 1)))
        blocks.append(per_tile)
        for b in range(B):
            for pt in range(NPT):
                lo = pt * 128
                hi = min(int(nval[b]), (pt + 1) * 128)
                for jb in per_tile[pt]:
                    gm = np.zeros((128, 128), np.float32)
                    if hi > lo:
                        p = np.arange(lo, hi)
                        v1 = s1[b, lo:hi]
                        w2 = lam[b, lo:hi]
                        w1 = np.float32(1.0) - w2
                        r1 = v1 - jb * 128
                        m1 = (r1 >= 0) & (r1 < 128)
                        np.add.at(gm, (r1[m1], p[m1] - lo), w1[m1])
                        r2 = v1 + 1 - jb * 128
                        m2 = (r2 >= 0) & (r2 < 128)
                        np.add.at(gm, (r2[m2], p[m2] - lo), w2[m2])
                    gdata[(l, b, pt, jb)] = gm
    return blocks, gdata


def _gate_perm():
    # torch gate order i,f,g,o -> ours i,f,o,g
    return np.concatenate([np.arange(0, 64), np.arange(96, 128), np.arange(64, 96)])


def _host_prepare(inputs):
    x = np.asarray(inputs["x"], np.float32)            # (B, L, DF0)
    scales_raw = np.asarray(inputs["scales_raw"], np.float32)
    len_seg = np.asarray(inputs["len_seg"])

    s1_all, lam_all, nval_all = [], [], []
    for l in range(3):
        s1, lam, nv = _interp_indices(scales_raw[l] + np.float32(0.5), len_seg[l])
        s1_all.append(s1); lam_all.append(lam); nval_all.append(nv)
    blocks, gdata = _build_g_blocks(s1_all, lam_all, nval_all)

    # conv weights: cw{l} flat (128 k, 2 mh x 10 kd x 128 m)
    conv_w = []
    for wname in ["w0", "w1", "w2"]:
        w = np.asarray(inputs[wname], np.float32)      # (256, Cin, 5)
        flat = np.zeros((128, 20 * 128), np.float32)
        for mh in range(2):
            for kc in range(2):
                for d in range(5):
                    kd = kc * 5 + d
                    blk = w[mh * 128:(mh + 1) * 128, kc * 128:(kc + 1) * 128, d].T
                    flat[:, (mh * 10 + kd) * 128:(mh * 10 + kd + 1) * 128] = blk
        conv_w.append(flat)
    w0 = np.asarray(inputs["w0"], np.float32)
    cw0x = np.zeros((5, 256), np.float32)
    for mh in range(2):
        cw0x[:, mh * 128:(mh + 1) * 128] = w0[mh * 128:(mh + 1) * 128, 256, :].T

    gam = np.zeros((128, 6), np.float32)
    bet = np.zeros((128, 6), np.float32)
    for l, (g, be) in enumerate([("g0", "be0"), ("g1", "be1"), ("g2", "be2")]):
        gv = np.asarray(inputs[g], np.float32)
        bv = np.asarray(inputs[be], np.float32)
        for mh in range(2):
            gam[:, l * 2 + mh] = gv[mh * 128:(mh + 1) * 128]
            bet[:, l * 2 + mh] = bv[mh * 128:(mh + 1) * 128]

    perm = _gate_perm()
    wih = np.zeros((128, 512), np.float32)   # col (d*2+kc)*128+m
    whh = np.zeros((128, 256), np.float32)   # col d*128+m; rows replicated
                                             # per 32-block (quadrant bases)
    for d, sfx in enumerate(["f", "b"]):
        wi = np.asarray(inputs[f"wih_{sfx}"], np.float32)[perm]   # (128, 256)
        wh = np.asarray(inputs[f"whh_{sfx}"], np.float32)[perm]   # (128, 32)
        # g-gate rows (96:128 after perm) pre-scaled 2x: tanh(g)=2*sig(2g)-1
        wi = wi.copy(); wh = wh.copy()
        wi[96:128] *= 2.0
        wh[96:128] *= 2.0
        for kc in range(2):
            wih[:, (d * 2 + kc) * 128:(d * 2 + kc + 1) * 128] = \
                wi[:, kc * 128:(kc + 1) * 128].T
        for q in range(4):
            whh[q * 32:(q + 1) * 32, d * 128:(d + 1) * 128] = wh.T
        bsum = (np.asarray(inputs[f"bih_{sfx}"], np.float32)
                + np.asarray(inputs[f"bhh_{sfx}"], np.float32))
        assert np.all(bsum == 0.0), "nonzero LSTM biases unsupported"

    xcm = np.transpose(x, (0, 2, 1))                    # (B, 257, L)
    nblk_layer = [sum(len(blocks[l][pt]) for pt in range(NPT)) for l in range(3)]
    in_maps = []
    for core in range(NCORES):
        sl = slice(core * SPC, (core + 1) * SPC)
        xp = np.zeros((SPC, DF0, XPAD), np.float32)
        xp[:, :, 2:2 + L] = xcm[sl]
        x5 = np.zeros((SPC, 5, XPAD), np.float32)
        ext = np.zeros((SPC, XPAD + 4), np.float32)
        ext[:, :XPAD] = xp[:, 256]
        for r in range(5):
            x5[:, r, :] = ext[:, r:r + XPAD]
        gl = []
        for l in range(3):
            for s in range(SPC):
                b = core * SPC + s
                for pt in range(NPT):
                    for jb in blocks[l][pt]:
                        gl.append(gdata[(l, b, pt, jb)])
        gblk = np.stack(gl)                              # (NBLK, 128, 128)
        gflat = gblk.transpose(1, 0, 2).reshape(128, -1)  # (128, NBLK*128)
        in_maps.append({
            "wz": np.zeros((128, 2), np.float32),
            "x": xp[:, :256].astype(bf16),
            "x5": x5.astype(bf16),
            "cw0": conv_w[0].astype(bf16), "cw0x": cw0x.astype(bf16),
            "cw1": conv_w[1].astype(bf16), "cw2": conv_w[2].astype(bf16),
            "gam": gam, "bet": bet,
            "gblk": gflat.astype(bf16),
            "wih": wih.astype(bf16), "whh": whh.astype(bf16),
            "ident": np.eye(128, dtype=bf16),
        })
    meta = {"blocks": blocks, "nblk_layer": nblk_layer,
            "nblk_total": sum(nblk_layer) * SPC}
    return in_maps, meta


# ---------------------------------------------------------------- device program

def _neg_ap(tile_ap, col0, step1, count1, step2, count2):
    """strided (possibly negative) 2-level free AP over a [128, N] tile."""
    ap = tile_ap.copy()
    p0 = list(ap.ap[0])
    ap.ap = bass_rust.VecI64Pair([p0, [step1, count1], [step2, count2]])
    ap.offset = ap.offset + col0
    return ap


def _build_program(meta, debug=False):
    blocks = meta["blocks"]
    nblk_layer = meta["nblk_layer"]

    nc = bass.Bass()
    if debug:
        dbg_seqs_d = nc.dram_tensor("dbg_seqs", [SPC, 2, 128, SPAD],
                                    dt.float16, kind="ExternalOutput")
        dbg_xg_d = nc.dram_tensor("dbg_xg", [128, 4 * SPAD],
                                  dt.float16, kind="ExternalOutput")
        dbg_y_d = nc.dram_tensor("dbg_y", [3, SPC, 2, 128, L], dt.float32,
                                 kind="ExternalOutput")
        dbg_int_d = nc.dram_tensor("dbg_int", [3, SPC, 2, 128, XPAD],
                                   dt.float16, kind="ExternalOutput")
        dbg_zt_d = nc.dram_tensor("dbg_zt", [3, SPC, 2, 128, NPT * 128],
                                  dt.float16, kind="ExternalOutput")
        dbg_gb_d = nc.dram_tensor("dbg_gb", [128, meta["nblk_total"] * 128],
                                  dt.float16, kind="ExternalOutput")
    wz_d = nc.dram_tensor("wz", [128, 2], dt.float32, kind="ExternalInput")
    x_d = nc.dram_tensor("x", [SPC, 256, XPAD], dt.float16, kind="ExternalInput")
    x5_d = nc.dram_tensor("x5", [SPC, 5, XPAD], dt.float16, kind="ExternalInput")
    cw_d = [nc.dram_tensor(f"cw{l}", [128, 20 * 128], dt.float16,
                           kind="ExternalInput") for l in range(3)]
    cw0x_d = nc.dram_tensor("cw0x", [5, 256], dt.float16, kind="ExternalInput")
    gam_d = nc.dram_tensor("gam", [128, 6], dt.float32, kind="ExternalInput")
    bet_d = nc.dram_tensor("bet", [128, 6], dt.float32, kind="ExternalInput")
    gblk_d = nc.dram_tensor("gblk", [128, meta["nblk_total"] * 128], dt.float16,
                            kind="ExternalInput")
    wih_d = nc.dram_tensor("wih", [128, 512], dt.float16, kind="ExternalInput")
    whh_d = nc.dram_tensor("whh", [128, 256], dt.float16, kind="ExternalInput")
    ident_d = nc.dram_tensor("ident", [128, 128], dt.float16, kind="ExternalInput")
    hout_d = nc.dram_tensor("hout", [128, 256], dt.float32,
                            kind="ExternalOutput")

    lay_off = [0, SPC * nblk_layer[0], SPC * (nblk_layer[0] + nblk_layer[1])]
    inv_n = 1.0 / (B * L)
    groups = [list(range(NCORES))]

    with tile.TileContext(nc) as tc:
        with (
            tc.tile_pool(name="const", bufs=1) as cp,
            tc.tile_pool(name="bufs", bufs=1) as bp,
            tc.tile_pool(name="dram", bufs=2, space="DRAM") as dp,
        ):
            # ---- warmup AllReduce: reads host-provided zeros directly from
            # DRAM (no on-chip deps), first on the gpsimd queue so the CC
            # trigger fires at engine start and the mesh absorbs core-start
            # skew while conv runs
            wsin = dp.tile([128, 2], dt.float32, tag="wcin", name="wcin")
            wsout = dp.tile([128, 2], dt.float32, tag="wcout", name="wcout")
            nc.sync.dma_start(wsin[:], wz_d[:])
            nc.gpsimd.collective_compute(
                "AllReduce", ALU.add, replica_groups=[list(range(NCORES))],
                ins=[wsin.opt()], outs=[wsout.opt()])
            # ---- constants: critical loads (x, cw0, x5) on scalar queue
            # first; everything else deferred onto the gpsimd queue
            xa = [[bp.tile([128, XPAD], dt.float16, tag=f"xa{s}{h}",
                           name=f"xa{s}{h}")
                   for h in range(2)] for s in range(SPC)]
            xb = [[bp.tile([128, XPAD], dt.float16, tag=f"xb{s}{h}",
                           name=f"xb{s}{h}")
                   for h in range(2)] for s in range(SPC)]
            x5t = [bp.tile([5, XPAD], dt.float16, tag=f"x5{s}", name=f"x5t{s}")
                   for s in range(SPC)]
            seqs = [[bp.tile([128, SPAD], dt.float16, tag=f"sq{s}{h}",
                             name=f"sq{s}{h}")
                     for h in range(2)] for s in range(SPC)]
            cw = [cp.tile([128, 20 * 128], dt.float16, tag=f"cw{l}",
                          name=f"cw{l}")
                  for l in range(3)]
            cw0x = cp.tile([5, 256], dt.float16)
            for h in range(2):
                nc.scalar.dma_start(xa[0][h][:], x_d[0, h * 128:(h + 1) * 128, :])
            nc.scalar.dma_start(cw[0][:], cw_d[0][:])
            nc.scalar.dma_start(cw0x[:], cw0x_d[:])
            nc.scalar.dma_start(x5t[0][:], x5_d[0])
            for h in range(2):
                nc.scalar.dma_start(xa[1][h][:], x_d[1, h * 128:(h + 1) * 128, :])
            nc.scalar.dma_start(x5t[1][:], x5_d[1])
            # preload ACT sigmoid/tanh tables during the idle startup so the
            # first LSTM activation doesn't eat the table-load latency
            warm = cp.tile([1, 2], dt.float32)
            nc.vector.memset(warm[:], 0.0)
            nc.scalar.activation(warm[:, 0:1], warm[:, 0:1], AF.Sigmoid)
            nc.scalar.activation(warm[:, 1:2], warm[:, 1:2], AF.Tanh)
            gam = cp.tile([128, 6], dt.float32)
            bet = cp.tile([128, 6], dt.float32)
            nc.gpsimd.dma_start(gam[:], gam_d[:])
            nc.gpsimd.dma_start(bet[:], bet_d[:])
            wih = cp.tile([128, 512], dt.float16)
            nc.gpsimd.dma_start(wih[:], wih_d[:])
            whh = cp.tile([128, 256], dt.float16)
            nc.gpsimd.dma_start(whh[:], whh_d[:])
            ident = cp.tile([128, 128], dt.float16)
            nc.gpsimd.dma_start(ident[:], ident_d[:])
            nc.gpsimd.dma_start(cw[1][:], cw_d[1][:])
            nc.gpsimd.dma_start(cw[2][:], cw_d[2][:])
            for s in range(SPC):
                for h in range(2):
                    nc.vector.memset(xb[s][h][:, 0:2], 0.0)
                    nc.vector.memset(xb[s][h][:, XPAD - 2:XPAD], 0.0)
                    nc.vector.memset(seqs[s][h][:, 0:PAD], 0.0)
                    nc.vector.memset(seqs[s][h][:, SPAD - PAD:SPAD], 0.0)

            # ================================ conv + interp layers
            with (
                tc.tile_pool(name="convbuf", bufs=1) as cvp,
                tc.tile_pool(name="scratch", bufs=2) as scr,
                tc.tile_pool(name="bnscr", bufs=1) as bns,
                tc.tile_pool(name="cpsum", bufs=4, space="PSUM") as cps,
                tc.tile_pool(name="ipsum", bufs=2, space="PSUM") as ipp,
                tc.tile_pool(name="tpsum", bufs=2, space="PSUM") as tpp,
            ):
                y = [[cvp.tile([128, L], dt.float32, tag=f"y{s}{h}",
                               name=f"y{s}{h}")
                      for h in range(2)] for s in range(SPC)]
                zt = [[cvp.tile([128, NPT * 128], dt.float16, tag=f"zt{s}{h}",
                                name=f"zt{s}{h}")
                       for h in range(2)] for s in range(SPC)]
                gbuf = cvp.tile([128, meta["nblk_total"] * 128], dt.float16,
                                tag="gb")
                # layer-0 G blocks load now; l1/l2 are deferred until after
                # the first stats AllReduce is triggered so its tiny input
                # DMA isn't queued behind megabytes of G-block descriptors
                a1_l0 = (lay_off[0] + SPC * nblk_layer[0]) * 128
                nc.gpsimd.dma_start(gbuf[:, 0:a1_l0], gblk_d[:, 0:a1_l0])
                sacc = cvp.tile([128, 16], dt.float32)
                qacc = cvp.tile([128, 16], dt.float32)
                stats = cvp.tile([128, 4], dt.float32)
                statsg = cvp.tile([128, 4], dt.float32)
                abt = cvp.tile([128, 4], dt.float32)
                t2 = cvp.tile([128, 2], dt.float32)
                epst = cvp.tile([128, 1], dt.float32)
                nc.vector.memset(epst[:], EPS)
                bnt = bns.tile([128, L // 2], dt.float32, tag="bnt")

                cur, nxt = xa, xb
                for l in range(3):
                    nkd = 11 if l == 0 else 10
                    per_pt_off = {}
                    off = 0
                    for pt in range(NPT):
                        per_pt_off[pt] = off
                        off += len(blocks[l][pt])

                    souts = []

                    def conv_bank(mh, s, lt):
                        ps = cps.tile([128, 512], dt.float32, tag="cps")
                        for kd in range(nkd):
                            if kd < 10:
                                lhs = cw[l][:, (mh * 10 + kd) * 128:
                                            (mh * 10 + kd + 1) * 128]
                                kc, d = divmod(kd, 5)
                                rhs = cur[s][kc][:, lt * 512 + d:
                                                 lt * 512 + d + 512]
                            else:
                                lhs = cw0x[:, mh * 128:(mh + 1) * 128]
                                rhs = x5t[s][:, lt * 512:lt * 512 + 512]
                            nc.tensor.matmul(ps[:], lhs, rhs,
                                             start=(kd == 0),
                                             stop=(kd == nkd - 1))
                        k = mh * 8 + s * 4 + lt
                        ysl = y[s][mh][:, lt * 512:(lt + 1) * 512]
                        nc.scalar.activation(ysl, ps[:], AF.Copy,
                                             accum_out=sacc[:, k:k + 1])
                        sq = scr.tile([128, 512], dt.float32, tag="sq")
                        nc.scalar.activation(sq[:], ps[:], AF.Square,
                                             accum_out=qacc[:, k:k + 1])

                    def emit_stats(mhs):
                        eng = nc.vector
                        for mh in mhs:
                            eng.tensor_reduce(
                                stats[:, 2 * mh:2 * mh + 1],
                                sacc[:, mh * 8:mh * 8 + 8],
                                mybir.AxisListType.X, ALU.add)
                            eng.tensor_reduce(
                                stats[:, 2 * mh + 1:2 * mh + 2],
                                qacc[:, mh * 8:mh * 8 + 8],
                                mybir.AxisListType.X, ALU.add)
                        w = 2 * len(mhs)
                        mh0 = mhs[0]
                        sin = dp.tile([128, w], dt.float32, tag=f"cin{w}",
                                      name=f"cin{l}{mh0}")
                        sout = dp.tile([128, w], dt.float32, tag=f"cout{w}",
                                       name=f"cout{l}{mh0}")
                        nc.scalar.dma_start(sin[:],
                                            stats[:, 2 * mh0:2 * mh0 + w])
                        nc.gpsimd.collective_compute(
                            "AllReduce", ALU.add, replica_groups=groups,
                            ins=[sin.opt()], outs=[sout.opt()])
                        souts.append(sout)
                        nc.sync.dma_start(statsg[:, 2 * mh0:2 * mh0 + w],
                                          sout[:])

                    def coef_pre(mh):
                        sm = statsg[:, 2 * mh:2 * mh + 1]
                        qm = statsg[:, 2 * mh + 1:2 * mh + 2]
                        nc.vector.scalar_tensor_tensor(
                            t2[:, mh:mh + 1], sm, inv_n, sm, ALU.mult, ALU.mult)
                        nc.vector.tensor_tensor(t2[:, mh:mh + 1], qm,
                                                t2[:, mh:mh + 1], ALU.subtract)

                    def coef_sqrt(mh):
                        nc.scalar.activation(t2[:, mh:mh + 1], t2[:, mh:mh + 1],
                                             AF.Sqrt, bias=epst[:], scale=inv_n)

                    def coef_post(mh):
                        sm = statsg[:, 2 * mh:2 * mh + 1]
                        nc.vector.reciprocal(t2[:, mh:mh + 1], t2[:, mh:mh + 1])
                        nc.vector.tensor_tensor(
                            abt[:, mh:mh + 1], gam[:, 2 * l + mh:2 * l + mh + 1],
                            t2[:, mh:mh + 1], ALU.mult)
                        nc.vector.scalar_tensor_tensor(
                            t2[:, mh:mh + 1], sm, inv_n, abt[:, mh:mh + 1],
                            ALU.mult, ALU.mult)
                        nc.vector.tensor_tensor(
                            abt[:, 2 + mh:3 + mh],
                            bet[:, 2 * l + mh:2 * l + mh + 1],
                            t2[:, mh:mh + 1], ALU.subtract)

                    def emit_interp(mh):
                        deng = nc.vector if mh == 0 else nc.scalar
                        for s in range(SPC):
                            sbase = lay_off[l] + s * nblk_layer[l]
                            for w in range(4):
                                pts = list(range(4 * w, 4 * w + 4))
                                psw = ipp.tile([128, 512], dt.float32,
                                               tag="ipw", name=f"ipw{w}")
                                for pt in pts:
                                    bl = blocks[l][pt]
                                    k = pt - 4 * w
                                    for jb in bl:
                                        lhs = zt[s][mh][:, jb * 128:
                                                        (jb + 1) * 128]
                                        gi = sbase + per_pt_off[pt] + bl.index(jb)
                                        rhs = gbuf[:, gi * 128:(gi + 1) * 128]
                                        nc.tensor.matmul(
                                            psw[:, k * 128:(k + 1) * 128],
                                            lhs, rhs,
                                            start=(jb == bl[0]),
                                            stop=(jb == bl[-1]))
                                if l < 2:
                                    dst = nxt[s][mh][:, 2 + 512 * w:
                                                     2 + 512 * (w + 1)]
                                else:
                                    dst = seqs[s][mh][:, PAD + 512 * w:
                                                      PAD + 512 * (w + 1)]
                                if mh == 0:
                                    deng.tensor_copy(dst, psw[:])
                                else:
                                    deng.copy(dst, psw[:])

                    def emit_transpose(mh):
                        # PE-transpose BN'd z [128ch, L] into position-major
                        # zt tiles; 4 tiles share one psum bank, one drain
                        deng = nc.vector if mh == 0 else nc.scalar
                        for s in range(SPC):
                            src = nxt[s][mh]
                            for w in range(4):
                                tp = tpp.tile([128, 512], dt.float16,
                                              tag="tp", name=f"tp{w}")
                                for k in range(4):
                                    pt = 4 * w + k
                                    nc.tensor.transpose(
                                        tp[:, k * 128:(k + 1) * 128],
                                        src[:, 2 + 128 * pt:2 + 128 * (pt + 1)],
                                        ident[:])
                                dst = zt[s][mh][:, 512 * w:512 * (w + 1)]
                                if mh == 0:
                                    deng.tensor_copy(dst, tp[:])
                                else:
                                    deng.copy(dst, tp[:])

                    # ---- conv mh0 (+ AR0 mid-conv for l>0; layer 0's first
                    # AR is core-skew-bound anyway, so it carries BOTH
                    # halves' stats after conv ends -> one mesh, not two)
                    for s in range(SPC):
                        for lt in range(4):
                            conv_bank(0, s, lt)
                    if l > 0:
                        emit_stats([0])
                        coef_pre(0)
                    # ---- conv mh1; sqrt0 interleaves into the scalar stream
                    # after bank 2 so it runs mid-conv-mh1 once AR0 lands
                    banks1 = [(s, lt) for s in range(SPC) for lt in range(4)]
                    for bi, (s, lt) in enumerate(banks1):
                        if bi == 3 and l > 0:
                            coef_sqrt(0)
                        conv_bank(1, s, lt)
                    if l == 0:
                        emit_stats([0, 1])
                        # deferred G-block loads: enqueue behind the AR
                        # trigger so the stats DMA isn't ring-blocked
                        a1_l2 = (lay_off[2] + SPC * nblk_layer[2]) * 128
                        nc.gpsimd.dma_start(gbuf[:, a1_l0:a1_l2],
                                            gblk_d[:, a1_l0:a1_l2])
                        coef_pre(0)
                        coef_sqrt(0)
                    coef_post(0)
                    # ---- BN0 split across DVE (s0) + ACT (s1) so the
                    # post-AR tail is short
                    for hf in range(2):
                        ysl = y[0][0][:, hf * 1024:(hf + 1) * 1024]
                        ztar = nxt[0][0][:, 2 + hf * 1024:2 + (hf + 1) * 1024]
                        nc.vector.tensor_scalar_mul(bnt[:], ysl, abt[:, 0:1])
                        nc.vector.tensor_scalar(ztar, bnt[:], abt[:, 2:3],
                                                0.0, ALU.add, ALU.max)
                    for hf in range(2):
                        ysl = y[1][0][:, hf * 1024:(hf + 1) * 1024]
                        ztar = nxt[1][0][:, 2 + hf * 1024:2 + (hf + 1) * 1024]
                        nc.scalar.activation(ztar, ysl, AF.Relu,
                                             bias=abt[:, 2:3],
                                             scale=abt[:, 0:1])
                    emit_transpose(0)
                    if l > 0:
                        emit_stats([1])
                    # ---- interp mh0 (hides AR1)
                    emit_interp(0)
                    # ---- coefs + BN1 on scalar at half-L granularity
                    coef_pre(1)
                    coef_sqrt(1)
                    coef_post(1)
                    for hf in range(2):
                        ysl = y[0][1][:, hf * 1024:(hf + 1) * 1024]
                        ztar = nxt[0][1][:, 2 + hf * 1024:2 + (hf + 1) * 1024]
                        nc.scalar.activation(ztar, ysl, AF.Relu,
                                             bias=abt[:, 3:4],
                                             scale=abt[:, 1:2])
                    for hf in range(2):
                        ysl = y[1][1][:, hf * 1024:(hf + 1) * 1024]
                        ztar = nxt[1][1][:, 2 + hf * 1024:2 + (hf + 1) * 1024]
                        nc.vector.tensor_scalar_mul(bnt[:], ysl, abt[:, 1:2])
                        nc.vector.tensor_scalar(ztar, bnt[:], abt[:, 3:4],
                                                0.0, ALU.add, ALU.max)
                    emit_transpose(1)
                    emit_interp(1)
                    if debug:
                        for s in range(SPC):
                            for h in range(2):
                                nc.sync.dma_start(dbg_y_d[l, s, h], y[s][h][:])
                                nc.sync.dma_start(dbg_zt_d[l, s, h],
                                                  zt[s][h][:])
                                if l < 2:
                                    nc.sync.dma_start(dbg_int_d[l, s, h],
                                                      nxt[s][h][:])
                        if l == 0:
                            nc.sync.dma_start(dbg_gb_d[:], gbuf[:])
                    if l < 2:
                        cur, nxt = nxt, cur

            # ================================ LSTM via Picard iteration
            # Quadrant q = d*2+s (d=dir, s=sample). Per iteration:
            #   G_q = Wih_d x_q (+ Whh_d h_prev_q shifted by 1) via matmuls
            #   sg = sigmoid(G) over all 128 gate rows (g rows pre-scaled
            #        2x on host: tanh(g) = 2 sig(2g) - 1)
            #   u  = 2 sg_i sg_g - sg_i               (DVE)
            #   c  = tensor_tensor_scan(sg_f, u)      (c_t = f c_{t-1} + u_t)
            #   h  = sg_o tanh(c)                     (ACT + DVE)
            # bwd quadrants read seqs reversed (manual AP) and are emitted
            # after all fwd matmuls so tensor program order covers the
            # untracked reads. h lives at col t+1 (col 0 = zeros) so the
            # Whh matmul for chunk c reads h_{t-1} as cols [c*512, c*512+512).
            NIT = 3
            CH = 512
            NCHK = L // CH
            with (
                tc.tile_pool(name="lstm", bufs=1) as lp,
                tc.tile_pool(name="work", bufs=2) as wp,
                tc.tile_pool(name="psg", bufs=4, space="PSUM") as gp,
                tc.tile_pool(name="pst", bufs=1, space="PSUM") as tp2,
            ):
                sgt = lp.tile([128, 4 * L], dt.float16, tag="sgt", name="sgt")
                # xg cached in fp16: computed by matmul once (it=0), replayed
                # into psum via identity matmul for later iterations' whh
                # accumulation (cheaper than recomputing the 2-block xg)
                xgs = lp.tile([128, 4 * L], dt.float16, tag="xgs", name="xgs")
                # quadrant-packed [128 = 4q x 32, L] gate planes: DVE ops are
                # column-bound regardless of rows, so every elementwise op
                # runs once on all 4 quadrants. DMA does the partition-
                # crossing repack (rings are idle here).
                gpk = [lp.tile([128, L], dt.float16, tag=f"gp{g}",
                               name=f"gpk{g}") for g in range(4)]  # i,f,o,g
                t1 = lp.tile([128, L], dt.float16, tag="t1", name="t1")
                upk = lp.tile([128, L], dt.float16, tag="upk", name="upk")
                # h_t at col t+1 (col 0 zero); whh is row-replicated so its
                # lhsT base matches the packed rhs base per quadrant
                hpk = lp.tile([128, L + 1], dt.float16, tag="hpk", name="hpk")
                # PE base partitions are limited to 0/32/64: quadrant 3
                # (base 96) gets a DMA-unpacked copy at base 0
                hq3 = lp.tile([32, L + 1], dt.float16, tag="hq3", name="hq3")
                cbuf = lp.tile([128, L], dt.float16, tag="cbuf", name="cbuf")
                # tanh(c) lands in PSUM so the h multiply (opk SBUF + tcb
                # PSUM) dodges the equal-base-partition rule
                tcb = tp2.tile([128, L], dt.float32, tag="tcb", name="tcb")
                hfin = lp.tile([128, 256], dt.float32, tag="hfin",
                               name="hfin")
                nc.vector.memset(hpk[:, 0:1], 0.0)

                for it in range(NIT):
                    for d in range(2):
                        for s in range(SPC):
                            q = d * 2 + s
                            for c in range(NCHK):
                                xsl = xgs[:, q * L + c * CH:
                                          q * L + (c + 1) * CH]
                                ps = gp.tile([128, CH], dt.float32, tag="pg",
                                             name=f"pg{q}{c}")
                                if it == 0:
                                    for half in range(CH // 512):
                                        c0 = c * CH + half * 512
                                        pssl = ps[:, half * 512:
                                                  (half + 1) * 512]
                                        for kc in range(2):
                                            lhs = wih[:, (d * 2 + kc) * 128:
                                                      (d * 2 + kc + 1) * 128]
                                            if d == 0:
                                                rhs = seqs[s][kc][
                                                    :, PAD + c0:
                                                    PAD + c0 + 512]
                                            else:
                                                rhs = _neg_ap(
                                                    seqs[s][kc][:],
                                                    PAD + (L - 1) - c0,
                                                    -1, 512, 0, 1)
                                            nc.tensor.matmul(
                                                pssl, lhs, rhs,
                                                start=(kc == 0),
                                                stop=(kc == 1))
                                    nc.vector.tensor_copy(xsl, ps[:])
                                else:
                                    if q < 3:
                                        wsl = whh[q * 32:(q + 1) * 32,
                                                  d * 128:(d + 1) * 128]
                                    else:
                                        wsl = whh[0:32,
                                                  d * 128:(d + 1) * 128]
                                    for half in range(CH // 512):
                                        c0 = c * CH + half * 512
                                        pssl = ps[:, half * 512:
                                                  (half + 1) * 512]
                                        nc.tensor.matmul(
                                            pssl, ident[:],
                                            xgs[:, q * L + c0:
                                                q * L + c0 + 512],
                                            start=True, stop=False)
                                        if q < 3:
                                            hsl = hpk[q * 32:(q + 1) * 32,
                                                      c0:c0 + 512]
                                        else:
                                            hsl = hq3[:, c0:c0 + 512]
                                        nc.tensor.matmul(pssl, wsl, hsl,
                                                         start=False,
                                                         stop=True)
                                nc.scalar.activation(
                                    sgt[:, q * L + c * CH:
                                        q * L + (c + 1) * CH],
                                    ps[:], AF.Sigmoid)
                            # repack this quadrant's gate planes right away
                            # (overlaps the next quadrant's matmuls)
                            qc = slice(q * L, (q + 1) * L)
                            for g, eng in ((3, nc.sync), (0, nc.gpsimd),
                                           (1, nc.sync), (2, nc.gpsimd)):
                                eng.dma_start(gpk[g][q * 32:(q + 1) * 32, :],
                                              sgt[g * 32:(g + 1) * 32, qc])
                    # u = sig(i) * (2*sig(2g) - 1), c = scan: f*c + u,
                    # h = sig(o) tanh(c); scan/tanh/h chunked so they
                    # pipeline down the chain
                    for hf in range(2):
                        cs = slice(hf * 1024, (hf + 1) * 1024)
                        nc.vector.tensor_scalar(t1[:, cs], gpk[3][:, cs],
                                                2.0, 1.0, ALU.mult,
                                                ALU.subtract)
                        nc.vector.tensor_tensor(upk[:, cs], t1[:, cs],
                                                gpk[0][:, cs], ALU.mult)
                    for c in range(L // 512):
                        cs = slice(c * 512, (c + 1) * 512)
                        init = 0.0 if c == 0 else cbuf[:, c * 512 - 1:c * 512]
                        nc.vector.tensor_tensor_scan(
                            cbuf[:, cs], gpk[1][:, cs], upk[:, cs],
                            init, ALU.mult, ALU.add)
                        nc.scalar.activation(tcb[:, cs], cbuf[:, cs], AF.Tanh)
                        nc.vector.tensor_tensor(
                            hpk[:, 1 + c * 512:1 + (c + 1) * 512],
                            gpk[2][:, cs], tcb[:, cs], ALU.mult)
                        if it < NIT - 1:
                            nc.gpsimd.dma_start(
                                hq3[:, 1 + c * 512:1 + (c + 1) * 512],
                                hpk[96:128, 1 + c * 512:1 + (c + 1) * 512])

                # fwd needs h at t=8j+7 (col 8j+8 of hpk); bwd (stored
                # reversed) needs h_rev[L-1-8j] (col L-8j).
                for q in range(4):
                    src = hpk[q * 32:(q + 1) * 32, :].copy()
                    p0 = list(src.ap[0])
                    if q < 2:
                        src.ap = bass_rust.VecI64Pair([p0, [8, 256]])
                        src.offset = src.offset + 8
                    else:
                        src.ap = bass_rust.VecI64Pair([p0, [-8, 256]])
                        src.offset = src.offset + L
                    nc.vector.tensor_copy(hfin[q * 32:(q + 1) * 32, :], src)
                nc.sync.dma_start(hout_d[:], hfin[:])
                if debug:
                    for s in range(SPC):
                        for h in range(2):
                            nc.sync.dma_start(dbg_seqs_d[s, h], seqs[s][h][:])

    return nc


# ---------------------------------------------------------------- entry point

def _gather(res):
    """hout (128, 256) per core (rows = (d*2+s)*32 + hdim, cols = output
    position j) -> full (B, 256, 64) output."""
    out = np.zeros((B, 256, 64), np.float32)
    for core in range(NCORES):
        ho = res.results[core]["hout"]          # (128, 256)
        for s in range(SPC):
            bidx = core * SPC + s
            out[bidx, :, 0:32] = ho[s * 32:(s + 1) * 32, :].T
            out[bidx, :, 32:64] = ho[(2 + s) * 32:(3 + s) * 32, :].T
    return out


def kernel(**inputs):
    in_maps, meta = _host_prepare(inputs)
    nc = _build_program(meta)
    _fix_excess_waits(nc)
    res = run_bass_kernel_spmd(nc, in_maps, list(range(NCORES)))
    return _gather(res)



# revision 63
# speedup vs baseline: 1.1207x; 1.0221x over previous
"""F0Encoder Trainium2 kernel: 3x(conv1d+BN+relu+InterpLnr) + biLSTM, 8-core data parallel.

Strategy (v3):
- data parallel: 2 samples per core; BN batch stats via tiny AllReduce.
  A dependency-free warmup AllReduce fires at engine start so the first
  mesh absorbs core-start skew while conv runs. Layer 0 (first real AR,
  skew-bound anyway) carries both channel-halves' stats in one AR; layers
  1/2 split per half so AR(mh0) rides under conv(mh1).
- conv1d as K-chunked fp16 matmuls, (s,lt)-outer so psum banks retire
  early; per-bank stats ride the ACT drains (Copy/Square accum_out), so
  each AR launches right after its half's banks drain
- BN+relu split across DVE/ACT per sample; z transposed to position-major
  on the PE (4 transposes share a psum bank, one wide drain) -- no DMA
  transposes, keeping the rings free for collective traffic
- InterpLnr as block-banded fp16 matmuls; 4 position-tiles accumulate in
  one [128,512] psum tile (pt-major so psum groups stay contiguous) with
  a single wide drain
- LSTM exactly (no chunk burn-in): Picard iteration on the h-feedback,
  NIT=3. Per iteration: G = Wih x (cached after it0, replayed into psum
  via identity matmul) + Whh h_prev; one 128-row sigmoid per chunk (g
  rows pre-scaled 2x: tanh(g)=2*sig(2g)-1); gates DMA-repacked quadrant-
  major ([128 = 4 quadrants x 32] partitions) so u/h are single
  column-bound DVE ops and the whole c recurrence c_t = f*c_{t-1} + u_t
  is a tensor_tensor_scan over [128, 2048]. whh is row-replicated per
  quadrant base so its matmul reads packed h directly (quadrant 3 at
  base 96 exceeds the PE base limit and gets a DMA-unpacked copy).
"""

import numpy as np

import concourse.bass as bass
import concourse.mybir as mybir
import concourse.tile as tile
from concourse.tile import add_dep_helper
import bass_rust
from concourse.bass_utils import run_bass_kernel_spmd

dt = mybir.dt
AF = mybir.ActivationFunctionType
ALU = mybir.AluOpType
bf16 = np.float16

B, L, DF0, DE, H = 16, 2048, 257, 256, 32
MIN_SEG, MAX_SEG = 19, 32
MNS = L // MIN_SEG + 1          # 108 segments per sample
L2 = MAX_SEG * 2                # 64
EPS = 1e-5

NCORES = 8
SPC = B // NCORES               # 2 samples per core
TC = 16                         # LSTM chunk body length
BURN = 12                       # burn-in steps
S = TC + BURN                   # 32 serial steps
NCH = L // TC                   # 128 chunks per (sample, dir)
NGRP = 2
CPG = NCH // NGRP               # 64 chunks per group per quadrant
NSEQ = 4 * CPG                  # 256 cols per group: (q = d*2+s) x chunk
SAMP_T = [BURN + 7, BURN + 15]  # sampled steps (outputs every 8)
NPT = L // 128                  # 16 position tiles

XPAD = L + 4                    # conv padded length
PAD = TC                        # seqs pad on both sides
SPAD = L + 2 * PAD              # 2080

_MAX_WAITS = 1


def _fix_excess_waits(nc, max_waits=_MAX_WAITS):
    """walrus codegen rejects >1 sem wait per instruction; split extras onto
    preceding same-engine NOPs."""
    ctr = 0
    for fn in nc.m.functions:
        for bb in fn.blocks:
            insts = bb.instructions
            i = 0
            while i < len(insts):
                inst = insts[i]
                si = getattr(inst, "sync_info", None)
                if si is not None and len(si.on_wait) > max_waits:
                    waits = list(si.on_wait)
                    inst.sync_info = mybir.SyncInfo(
                        on_wait=waits[-max_waits:], on_update=list(si.on_update)
                    )
                    extra = waits[:-max_waits]
                    pos = i
                    for j in range(0, len(extra), max_waits):
                        nop = mybir.InstNoOp(name=f"wsplit_{ctr}", engine=inst.engine)
                        ctr += 1
                        nop.sync_info = mybir.SyncInfo(
                            on_wait=extra[j:j + max_waits], on_update=[]
                        )
                        insts.insert(pos, nop)
                        pos += 1
                        i += 1
                i += 1
    return ctr


# ---------------------------------------------------------------- host precompute

def _interp_indices(scales, lens):
    """Replicate reference interp_lnr index math in fp32.
    scales, lens: (B*MNS,) -> s1 (B,L) int64, lam (B,L) f32, nvalid (B,)"""
    scales = scales.reshape(B, MNS).astype(np.float32)
    lens = lens.reshape(B, MNS).astype(np.int64)
    s1 = np.zeros((B, L), np.int64)
    lam = np.zeros((B, L), np.float32)
    nval = np.zeros(B, np.int64)
    idx = np.arange(L2, dtype=np.float32)
    for b in range(B):
        pos = 0
        off = 0
        for g in range(MNS):
            sc = scales[b, g]
            ln = int(lens[b, g])
            isc = idx / sc                      # f32 division, as reference
            ifl = np.floor(isc)
            lm = isc - ifl
            ifl_i = ifl.astype(np.int64)
            m = (ifl < np.float32(ln - 1)) & ((ifl + np.float32(off)) < np.float32(L - 1))
            k = int(m.sum())
            take = min(k, L - pos)
            if take > 0:
                s1[b, pos:pos + take] = ifl_i[m][:take] + off
                lam[b, pos:pos + take] = lm[m][:take]
            pos += take
            off += ln
            if pos >= L:
                break
        nval[b] = pos
    return s1, lam, nval


def _build_g_blocks(s1_all, lam_all, nval_all):
    """blocks[l][pt] = union list of j-blocks over the whole batch (same for all
    cores -> one SPMD program); gdata[(l,b,pt,jb)] = (128,128) f32 G^T block."""
    blocks = []
    gdata = {}
    for l in range(3):
        s1 = s1_all[l]; lam = lam_all[l]; nval = nval_all[l]
        per_tile = []
        for pt in range(NPT):
            jset = set()
            for b in range(B):
                lo = pt * 128
                hi = min(int(nval[b]), (pt + 1) * 128)
                if hi <= lo:
                    continue
                v1 = s1[b, lo:hi]
                jset.add(int(v1.min()) // 128)
                jset.add((int(v1.max()) + 1) // 128)
            if not jset:
                jset = {min(pt, NPT - 1)}
            jlo, jhi = min(jset), min(max(jset), NPT - 1)
            per_tile.append(list(range(jlo, jhi + 1)))
        blocks.append(per_tile)
        for b in range(B):
            for pt in range(NPT):
                lo = pt * 128
                hi = min(int(nval[b]), (pt + 1) * 128)
                for jb in per_tile[pt]:
                    gm = np.zeros((128, 128), np.float32)
                    if hi > lo:
                        p = np.arange(lo, hi)
                        v1 = s1[b, lo:hi]
                        w2 = lam[b, lo:hi]
                        w1 = np.float32(1.0) - w2
                        r1 = v1 - jb * 128
                        m1 = (r1 >= 0) & (r1 < 128)
                        np.add.at(gm, (r1[m1], p[m1] - lo), w1[m1])
                        r2 = v1 + 1 - jb * 128
                        m2 = (r2 >= 0) & (r2 < 128)
                        np.add.at(gm, (r2[m2], p[m2] - lo), w2[m2])
                    gdata[(l, b, pt, jb)] = gm
    return blocks, gdata


def _gate_perm():
    # torch gate order i,f,g,o -> ours i,f,o,g
    return np.concatenate([np.arange(0, 64), np.arange(96, 128), np.arange(64, 96)])


def _host_prepare(inputs):
    x = np.asarray(inputs["x"], np.float32)            # (B, L, DF0)
    scales_raw = np.asarray(inputs["scales_raw"], np.float32)
    len_seg = np.asarray(inputs["len_seg"])

    s1_all, lam_all, nval_all = [], [], []
    for l in range(3):
        s1, lam, nv = _interp_indices(scales_raw[l] + np.float32(0.5), len_seg[l])
        s1_all.append(s1); lam_all.append(lam); nval_all.append(nv)
    blocks, gdata = _build_g_blocks(s1_all, lam_all, nval_all)

    # conv weights: cw{l} flat (128 k, 2 mh x 10 kd x 128 m)
    conv_w = []
    for wname in ["w0", "w1", "w2"]:
        w = np.asarray(inputs[wname], np.float32)      # (256, Cin, 5)
        flat = np.zeros((128, 20 * 128), np.float32)
        for mh in range(2):
            for kc in range(2):
                for d in range(5):
                    kd = kc * 5 + d
                    blk = w[mh * 128:(mh + 1) * 128, kc * 128:(kc + 1) * 128, d].T
                    flat[:, (mh * 10 + kd) * 128:(mh * 10 + kd + 1) * 128] = blk
        conv_w.append(flat)
    w0 = np.asarray(inputs["w0"], np.float32)
    cw0x = np.zeros((5, 256), np.float32)
    for mh in range(2):
        cw0x[:, mh * 128:(mh + 1) * 128] = w0[mh * 128:(mh + 1) * 128, 256, :].T

    gam = np.zeros((128, 6), np.float32)
    bet = np.zeros((128, 6), np.float32)
    for l, (g, be) in enumerate([("g0", "be0"), ("g1", "be1"), ("g2", "be2")]):
        gv = np.asarray(inputs[g], np.float32)
        bv = np.asarray(inputs[be], np.float32)
        for mh in range(2):
            gam[:, l * 2 + mh] = gv[mh * 128:(mh + 1) * 128]
            bet[:, l * 2 + mh] = bv[mh * 128:(mh + 1) * 128]

    perm = _gate_perm()
    wih = np.zeros((128, 512), np.float32)   # col (d*2+kc)*128+m
    whh = np.zeros((128, 256), np.float32)   # col d*128+m; rows replicated
                                             # per 32-block (quadrant bases)
    for d, sfx in enumerate(["f", "b"]):
        wi = np.asarray(inputs[f"wih_{sfx}"], np.float32)[perm]   # (128, 256)
        wh = np.asarray(inputs[f"whh_{sfx}"], np.float32)[perm]   # (128, 32)
        # g-gate rows (96:128 after perm) pre-scaled 2x: tanh(g)=2*sig(2g)-1
        wi = wi.copy(); wh = wh.copy()
        wi[96:128] *= 2.0
        wh[96:128] *= 2.0
        for kc in range(2):
            wih[:, (d * 2 + kc) * 128:(d * 2 + kc + 1) * 128] = \
                wi[:, kc * 128:(kc + 1) * 128].T
        for q in range(4):
            whh[q * 32:(q + 1) * 32, d * 128:(d + 1) * 128] = wh.T
        bsum = (np.asarray(inputs[f"bih_{sfx}"], np.float32)
                + np.asarray(inputs[f"bhh_{sfx}"], np.float32))
        assert np.all(bsum == 0.0), "nonzero LSTM biases unsupported"

    xcm = np.transpose(x, (0, 2, 1))                    # (B, 257, L)
    nblk_layer = [sum(len(blocks[l][pt]) for pt in range(NPT)) for l in range(3)]
    in_maps = []
    for core in range(NCORES):
        sl = slice(core * SPC, (core + 1) * SPC)
        xp = np.zeros((SPC, DF0, XPAD), np.float32)
        xp[:, :, 2:2 + L] = xcm[sl]
        x5 = np.zeros((SPC, 5, XPAD), np.float32)
        ext = np.zeros((SPC, XPAD + 4), np.float32)
        ext[:, :XPAD] = xp[:, 256]
        for r in range(5):
            x5[:, r, :] = ext[:, r:r + XPAD]
        gl = []
        for l in range(3):
            for s in range(SPC):
                b = core * SPC + s
                for pt in range(NPT):
                    for jb in blocks[l][pt]:
                        gl.append(gdata[(l, b, pt, jb)])
        gblk = np.stack(gl)                              # (NBLK, 128, 128)
        gflat = gblk.transpose(1, 0, 2).reshape(128, -1)  # (128, NBLK*128)
        in_maps.append({
            "wz": np.zeros((128, 2), np.float32),
            "x": xp[:, :256].astype(bf16),
            "x5": x5.astype(bf16),
            "cw0": conv_w[0].astype(bf16), "cw0x": cw0x.astype(bf16),
            "cw1": conv_w[1].astype(bf16), "cw2": conv_w[2].astype(bf16),
            "gam": gam, "bet": bet,
            "gblk": gflat.astype(bf16),
            "wih": wih.astype(bf16), "whh": whh.astype(bf16),
            "ident": np.eye(128, dtype=bf16),
        })
    meta = {"blocks": blocks, "nblk_layer": nblk_layer,
            "nblk_total": sum(nblk_layer) * SPC}
    return in_maps, meta


# ---------------------------------------------------------------- device program

def _neg_ap(tile_ap, col0, step1, count1, step2, count2):
    """strided (possibly negative) 2-level free AP over a [128, N] tile."""
    ap = tile_ap.copy()
    p0 = list(ap.ap[0])
    ap.ap = bass_rust.VecI64Pair([p0, [step1, count1], [step2, count2]])
    ap.offset = ap.offset + col0
    return ap


def _build_program(meta, debug=False):
    blocks = meta["blocks"]
    nblk_layer = meta["nblk_layer"]

    nc = bass.Bass()
    if debug:
        dbg_seqs_d = nc.dram_tensor("dbg_seqs", [SPC, 2, 128, SPAD],
                                    dt.float16, kind="ExternalOutput")
        dbg_xg_d = nc.dram_tensor("dbg_xg", [128, 4 * SPAD],
                                  dt.float16, kind="ExternalOutput")
        dbg_y_d = nc.dram_tensor("dbg_y", [3, SPC, 2, 128, L], dt.float32,
                                 kind="ExternalOutput")
        dbg_int_d = nc.dram_tensor("dbg_int", [3, SPC, 2, 128, XPAD],
                                   dt.float16, kind="ExternalOutput")
        dbg_zt_d = nc.dram_tensor("dbg_zt", [3, SPC, 2, 128, NPT * 128],
                                  dt.float16, kind="ExternalOutput")
        dbg_gb_d = nc.dram_tensor("dbg_gb", [128, meta["nblk_total"] * 128],
                                  dt.float16, kind="ExternalOutput")
    wz_d = nc.dram_tensor("wz", [128, 2], dt.float32, kind="ExternalInput")
    x_d = nc.dram_tensor("x", [SPC, 256, XPAD], dt.float16, kind="ExternalInput")
    x5_d = nc.dram_tensor("x5", [SPC, 5, XPAD], dt.float16, kind="ExternalInput")
    cw_d = [nc.dram_tensor(f"cw{l}", [128, 20 * 128], dt.float16,
                           kind="ExternalInput") for l in range(3)]
    cw0x_d = nc.dram_tensor("cw0x", [5, 256], dt.float16, kind="ExternalInput")
    gam_d = nc.dram_tensor("gam", [128, 6], dt.float32, kind="ExternalInput")
    bet_d = nc.dram_tensor("bet", [128, 6], dt.float32, kind="ExternalInput")
    gblk_d = nc.dram_tensor("gblk", [128, meta["nblk_total"] * 128], dt.float16,
                            kind="ExternalInput")
    wih_d = nc.dram_tensor("wih", [128, 512], dt.float16, kind="ExternalInput")
    whh_d = nc.dram_tensor("whh", [128, 256], dt.float16, kind="ExternalInput")
    ident_d = nc.dram_tensor("ident", [128, 128], dt.float16, kind="ExternalInput")
    hout_d = nc.dram_tensor("hout", [128, 256], dt.float32,
                            kind="ExternalOutput")

    lay_off = [0, SPC * nblk_layer[0], SPC * (nblk_layer[0] + nblk_layer[1])]
    inv_n = 1.0 / (B * L)
    groups = [list(range(NCORES))]

    with tile.TileContext(nc) as tc:
        with (
            tc.tile_pool(name="const", bufs=1) as cp,
            tc.tile_pool(name="bufs", bufs=1) as bp,
            tc.tile_pool(name="dram", bufs=2, space="DRAM") as dp,
        ):
            # ---- warmup AllReduce: reads host-provided zeros directly from
            # DRAM (no on-chip deps), first on the gpsimd queue so the CC
            # trigger fires at engine start and the mesh absorbs core-start
            # skew while conv runs
            wsin = dp.tile([128, 2], dt.float32, tag="wcin", name="wcin")
            wsout = dp.tile([128, 2], dt.float32, tag="wcout", name="wcout")
            nc.sync.dma_start(wsin[:], wz_d[:])
            nc.gpsimd.collective_compute(
                "AllReduce", ALU.add, replica_groups=[list(range(NCORES))],
                ins=[wsin.opt()], outs=[wsout.opt()])
            # ---- constants: critical loads (x, cw0, x5) on scalar queue
            # first; everything else deferred onto the gpsimd queue
            xa = [[bp.tile([128, XPAD], dt.float16, tag=f"xa{s}{h}",
                           name=f"xa{s}{h}")
                   for h in range(2)] for s in range(SPC)]
            xb = [[bp.tile([128, XPAD], dt.float16, tag=f"xb{s}{h}",
                           name=f"xb{s}{h}")
                   for h in range(2)] for s in range(SPC)]
            x5t = [bp.tile([5, XPAD], dt.float16, tag=f"x5{s}", name=f"x5t{s}")
                   for s in range(SPC)]
            seqs = [[bp.tile([128, SPAD], dt.float16, tag=f"sq{s}{h}",
                             name=f"sq{s}{h}")
                     for h in range(2)] for s in range(SPC)]
            cw = [cp.tile([128, 20 * 128], dt.float16, tag=f"cw{l}",
                          name=f"cw{l}")
                  for l in range(3)]
            cw0x = cp.tile([5, 256], dt.float16)
            for h in range(2):
                nc.scalar.dma_start(xa[0][h][:], x_d[0, h * 128:(h + 1) * 128, :])
            nc.scalar.dma_start(cw[0][:], cw_d[0][:])
            nc.scalar.dma_start(cw0x[:], cw0x_d[:])
            nc.scalar.dma_start(x5t[0][:], x5_d[0])
            for h in range(2):
                nc.scalar.dma_start(xa[1][h][:], x_d[1, h * 128:(h + 1) * 128, :])
            nc.scalar.dma_start(x5t[1][:], x5_d[1])
            # preload ACT sigmoid/tanh tables during the idle startup so the
            # first LSTM activation doesn't eat the table-load latency
            warm = cp.tile([1, 2], dt.float32)
            nc.vector.memset(warm[:], 0.0)
            nc.scalar.activation(warm[:, 0:1], warm[:, 0:1], AF.Sigmoid)
            nc.scalar.activation(warm[:, 1:2], warm[:, 1:2], AF.Tanh)
            gam = cp.tile([128, 6], dt.float32)
            bet = cp.tile([128, 6], dt.float32)
            nc.gpsimd.dma_start(gam[:], gam_d[:])
            nc.gpsimd.dma_start(bet[:], bet_d[:])
            wih = cp.tile([128, 512], dt.float16)
            nc.gpsimd.dma_start(wih[:], wih_d[:])
            whh = cp.tile([128, 256], dt.float16)
            nc.gpsimd.dma_start(whh[:], whh_d[:])
            ident = cp.tile([128, 128], dt.float16)
            nc.gpsimd.dma_start(ident[:], ident_d[:])
            nc.gpsimd.dma_start(cw[1][:], cw_d[1][:])
            nc.gpsimd.dma_start(cw[2][:], cw_d[2][:])
            for s in range(SPC):
                for h in range(2):
                    nc.vector.memset(xb[s][h][:, 0:2], 0.0)
                    nc.vector.memset(xb[s][h][:, XPAD - 2:XPAD], 0.0)
                    nc.vector.memset(seqs[s][h][:, 0:PAD], 0.0)
                    nc.vector.memset(seqs[s][h][:, SPAD - PAD:SPAD], 0.0)

            # ================================ conv + interp layers
            with (
                tc.tile_pool(name="convbuf", bufs=1) as cvp,
                tc.tile_pool(name="scratch", bufs=2) as scr,
                tc.tile_pool(name="bnscr", bufs=1) as bns,
                tc.tile_pool(name="cpsum", bufs=4, space="PSUM") as cps,
                tc.tile_pool(name="ipsum", bufs=2, space="PSUM") as ipp,
                tc.tile_pool(name="tpsum", bufs=2, space="PSUM") as tpp,
            ):
                y = [[cvp.tile([128, L], dt.float32, tag=f"y{s}{h}",
                               name=f"y{s}{h}")
                      for h in range(2)] for s in range(SPC)]
                zt = [[cvp.tile([128, NPT * 128], dt.float16, tag=f"zt{s}{h}",
                                name=f"zt{s}{h}")
                       for h in range(2)] for s in range(SPC)]
                gbuf = cvp.tile([128, meta["nblk_total"] * 128], dt.float16,
                                tag="gb")
                # layer-0 G blocks load now; l1/l2 are deferred until after
                # the first stats AllReduce is triggered so its tiny input
                # DMA isn't queued behind megabytes of G-block descriptors
                a1_l0 = (lay_off[0] + SPC * nblk_layer[0]) * 128
                nc.gpsimd.dma_start(gbuf[:, 0:a1_l0], gblk_d[:, 0:a1_l0])
                sacc = cvp.tile([128, 16], dt.float32)
                qacc = cvp.tile([128, 16], dt.float32)
                stats = cvp.tile([128, 4], dt.float32)
                statsg = cvp.tile([128, 4], dt.float32)
                abt = cvp.tile([128, 4], dt.float32)
                t2 = cvp.tile([128, 2], dt.float32)
                epst = cvp.tile([128, 1], dt.float32)
                nc.vector.memset(epst[:], EPS)
                bnt = bns.tile([128, L // 2], dt.float32, tag="bnt")

                cur, nxt = xa, xb
                for l in range(3):
                    nkd = 11 if l == 0 else 10
                    per_pt_off = {}
                    off = 0
                    for pt in range(NPT):
                        per_pt_off[pt] = off
                        off += len(blocks[l][pt])

                    souts = []

                    def conv_bank(mh, s, lt):
                        ps = cps.tile([128, 512], dt.float32, tag="cps")
                        for kd in range(nkd):
                            if kd < 10:
                                lhs = cw[l][:, (mh * 10 + kd) * 128:
                                            (mh * 10 + kd + 1) * 128]
                                kc, d = divmod(kd, 5)
                                rhs = cur[s][kc][:, lt * 512 + d:
                                                 lt * 512 + d + 512]
                            else:
                                lhs = cw0x[:, mh * 128:(mh + 1) * 128]
                                rhs = x5t[s][:, lt * 512:lt * 512 + 512]
                            nc.tensor.matmul(ps[:], lhs, rhs,
                                             start=(kd == 0),
                                             stop=(kd == nkd - 1))
                        k = mh * 8 + s * 4 + lt
                        ysl = y[s][mh][:, lt * 512:(lt + 1) * 512]
                        nc.scalar.activation(ysl, ps[:], AF.Copy,
                                             accum_out=sacc[:, k:k + 1])
                        sq = scr.tile([128, 512], dt.float32, tag="sq")
                        nc.scalar.activation(sq[:], ps[:], AF.Square,
                                             accum_out=qacc[:, k:k + 1])

                    def emit_stats(mhs):
                        eng = nc.vector
                        for mh in mhs:
                            eng.tensor_reduce(
                                stats[:, 2 * mh:2 * mh + 1],
                                sacc[:, mh * 8:mh * 8 + 8],
                                mybir.AxisListType.X, ALU.add)
                            eng.tensor_reduce(
                                stats[:, 2 * mh + 1:2 * mh + 2],
                                qacc[:, mh * 8:mh * 8 + 8],
                                mybir.AxisListType.X, ALU.add)
                        w = 2 * len(mhs)
                        mh0 = mhs[0]
                        sin = dp.tile([128, w], dt.float32, tag=f"cin{w}",
                                      name=f"cin{l}{mh0}")
                        sout = dp.tile([128, w], dt.float32, tag=f"cout{w}",
                                       name=f"cout{l}{mh0}")
                        nc.scalar.dma_start(sin[:],
                                            stats[:, 2 * mh0:2 * mh0 + w])
                        nc.gpsimd.collective_compute(
                            "AllReduce", ALU.add, replica_groups=groups,
                            ins=[sin.opt()], outs=[sout.opt()])
                        souts.append(sout)
                        nc.sync.dma_start(statsg[:, 2 * mh0:2 * mh0 + w],
                                          sout[:])

                    def coef_pre(mh):
                        sm = statsg[:, 2 * mh:2 * mh + 1]
                        qm = statsg[:, 2 * mh + 1:2 * mh + 2]
                        nc.vector.scalar_tensor_tensor(
                            t2[:, mh:mh + 1], sm, inv_n, sm, ALU.mult, ALU.mult)
                        nc.vector.tensor_tensor(t2[:, mh:mh + 1], qm,
                                                t2[:, mh:mh + 1], ALU.subtract)

                    def coef_sqrt(mh):
                        nc.scalar.activation(t2[:, mh:mh + 1], t2[:, mh:mh + 1],
                                             AF.Sqrt, bias=epst[:], scale=inv_n)

                    def coef_post(mh):
                        sm = statsg[:, 2 * mh:2 * mh + 1]
                        nc.vector.reciprocal(t2[:, mh:mh + 1], t2[:, mh:mh + 1])
                        nc.vector.tensor_tensor(
                            abt[:, mh:mh + 1], gam[:, 2 * l + mh:2 * l + mh + 1],
                            t2[:, mh:mh + 1], ALU.mult)
                        nc.vector.scalar_tensor_tensor(
                            t2[:, mh:mh + 1], sm, inv_n, abt[:, mh:mh + 1],
                            ALU.mult, ALU.mult)
                        nc.vector.tensor_tensor(
                            abt[:, 2 + mh:3 + mh],
                            bet[:, 2 * l + mh:2 * l + mh + 1],
                            t2[:, mh:mh + 1], ALU.subtract)

                    def emit_interp(mh):
                        deng = nc.vector if mh == 0 else nc.scalar
                        for s in range(SPC):
                            sbase = lay_off[l] + s * nblk_layer[l]
                            for w in range(4):
                                pts = list(range(4 * w, 4 * w + 4))
                                psw = ipp.tile([128, 512], dt.float32,
                                               tag="ipw", name=f"ipw{w}")
                                for pt in pts:
                                    bl = blocks[l][pt]
                                    k = pt - 4 * w
                                    for jb in bl:
                                        lhs = zt[s][mh][:, jb * 128:
                                                        (jb + 1) * 128]
                                        gi = sbase + per_pt_off[pt] + bl.index(jb)
                                        rhs = gbuf[:, gi * 128:(gi + 1) * 128]
                                        nc.tensor.matmul(
                                            psw[:, k * 128:(k + 1) * 128],
                                            lhs, rhs,
                                            start=(jb == bl[0]),
                                            stop=(jb == bl[-1]))
                                if l < 2:
                                    dst = nxt[s][mh][:, 2 + 512 * w:
                                                     2 + 512 * (w + 1)]
                                else:
                                    dst = seqs[s][mh][:, PAD + 512 * w:
                                                      PAD + 512 * (w + 1)]
                                if mh == 0:
                                    deng.tensor_copy(dst, psw[:])
                                else:
                                    deng.copy(dst, psw[:])

                    def emit_transpose(mh):
                        # PE-transpose BN'd z [128ch, L] into position-major
                        # zt tiles; 4 tiles share one psum bank, one drain
                        deng = nc.vector if mh == 0 else nc.scalar
                        for s in range(SPC):
                            src = nxt[s][mh]
                            for w in range(4):
                                tp = tpp.tile([128, 512], dt.float16,
                                              tag="tp", name=f"tp{w}")
                                for k in range(4):
                                    pt = 4 * w + k
                                    nc.tensor.transpose(
                                        tp[:, k * 128:(k + 1) * 128],
                                        src[:, 2 + 128 * pt:2 + 128 * (pt + 1)],
                                        ident[:])
                                dst = zt[s][mh][:, 512 * w:512 * (w + 1)]
                                if mh == 0:
                                    deng.tensor_copy(dst, tp[:])
                                else:
                                    deng.copy(dst, tp[:])

                    # ---- conv mh0 (+ AR0 mid-conv for l>0; layer 0's first
                    # AR is core-skew-bound anyway, so it carries BOTH
                    # halves' stats after conv ends -> one mesh, not two)
                    for s in range(SPC):
                        for lt in range(4):
                            conv_bank(0, s, lt)
                    if l > 0:
                        emit_stats([0])
                        coef_pre(0)
                    # ---- conv mh1; sqrt0 interleaves into the scalar stream
                    # after bank 2 so it runs mid-conv-mh1 once AR0 lands
                    banks1 = [(s, lt) for s in range(SPC) for lt in range(4)]
                    for bi, (s, lt) in enumerate(banks1):
                        if bi == 3 and l > 0:
                            coef_sqrt(0)
                        conv_bank(1, s, lt)
                    if l == 0:
                        emit_stats([0, 1])
                        # deferred G-block loads: enqueue behind the AR
                        # trigger so the stats DMA isn't ring-blocked
                        a1_l2 = (lay_off[2] + SPC * nblk_layer[2]) * 128
                        nc.gpsimd.dma_start(gbuf[:, a1_l0:a1_l2],
                                            gblk_d[:, a1_l0:a1_l2])
                        coef_pre(0)
                        coef_sqrt(0)
                    coef_post(0)
                    # ---- BN0 split across DVE (s0) + ACT (s1) so the
                    # post-AR tail is short
                    for hf in range(2):
                        ysl = y[0][0][:, hf * 1024:(hf + 1) * 1024]
                        ztar = nxt[0][0][:, 2 + hf * 1024:2 + (hf + 1) * 1024]
                        nc.vector.tensor_scalar_mul(bnt[:], ysl, abt[:, 0:1])
                        nc.vector.tensor_scalar(ztar, bnt[:], abt[:, 2:3],
                                                0.0, ALU.add, ALU.max)
                    for hf in range(2):
                        ysl = y[1][0][:, hf * 1024:(hf + 1) * 1024]
                        ztar = nxt[1][0][:, 2 + hf * 1024:2 + (hf + 1) * 1024]
                        nc.scalar.activation(ztar, ysl, AF.Relu,
                                             bias=abt[:, 2:3],
                                             scale=abt[:, 0:1])
                    emit_transpose(0)
                    if l > 0:
                        emit_stats([1])
                    # ---- interp mh0 (hides AR1)
                    emit_interp(0)
                    # ---- coefs + BN1 on scalar at half-L granularity
                    coef_pre(1)
                    coef_sqrt(1)
                    coef_post(1)
                    for hf in range(2):
                        ysl = y[0][1][:, hf * 1024:(hf + 1) * 1024]
                        ztar = nxt[0][1][:, 2 + hf * 1024:2 + (hf + 1) * 1024]
                        nc.scalar.activation(ztar, ysl, AF.Relu,
                                             bias=abt[:, 3:4],
                                             scale=abt[:, 1:2])
                    for hf in range(2):
                        ysl = y[1][1][:, hf * 1024:(hf + 1) * 1024]
                        ztar = nxt[1][1][:, 2 + hf * 1024:2 + (hf + 1) * 1024]
                        nc.vector.tensor_scalar_mul(bnt[:], ysl, abt[:, 1:2])
                        nc.vector.tensor_scalar(ztar, bnt[:], abt[:, 3:4],
                                                0.0, ALU.add, ALU.max)
                    emit_transpose(1)
                    emit_interp(1)
                    if debug:
                        for s in range(SPC):
                            for h in range(2):
                                nc.sync.dma_start(dbg_y_d[l, s, h], y[s][h][:])
                                nc.sync.dma_start(dbg_zt_d[l, s, h],
                                                  zt[s][h][:])
                                if l < 2:
                                    nc.sync.dma_start(dbg_int_d[l, s, h],
                                                      nxt[s][h][:])
                        if l == 0:
                            nc.sync.dma_start(dbg_gb_d[:], gbuf[:])
                    if l < 2:
                        cur, nxt = nxt, cur

            # ================================ LSTM via Picard iteration
            # Quadrant q = d*2+s (d=dir, s=sample). Per iteration:
            #   G_q = Wih_d x_q (+ Whh_d h_prev_q shifted by 1) via matmuls
            #   sg = sigmoid(G) over all 128 gate rows (g rows pre-scaled
            #        2x on host: tanh(g) = 2 sig(2g) - 1)
            #   u  = 2 sg_i sg_g - sg_i               (DVE)
            #   c  = tensor_tensor_scan(sg_f, u)      (c_t = f c_{t-1} + u_t)
            #   h  = sg_o tanh(c)                     (ACT + DVE)
            # bwd quadrants read seqs reversed (manual AP) and are emitted
            # after all fwd matmuls so tensor program order covers the
            # untracked reads. h lives at col t+1 (col 0 = zeros) so the
            # Whh matmul for chunk c reads h_{t-1} as cols [c*512, c*512+512).
            NIT = 3
            CH = 512
            NCHK = L // CH
            with (
                tc.tile_pool(name="lstm", bufs=1) as lp,
                tc.tile_pool(name="work", bufs=2) as wp,
                tc.tile_pool(name="psg", bufs=4, space="PSUM") as gp,
                tc.tile_pool(name="pst", bufs=1, space="PSUM") as tp2,
            ):
                sgt = lp.tile([128, 4 * L], dt.float16, tag="sgt", name="sgt")
                # xg cached in fp16: computed by matmul once (it=0), replayed
                # into psum via identity matmul for later iterations' whh
                # accumulation (cheaper than recomputing the 2-block xg)
                xgs = lp.tile([128, 4 * L], dt.float16, tag="xgs", name="xgs")
                # quadrant-packed [128 = 4q x 32, L] gate planes: DVE ops are
                # column-bound regardless of rows, so every elementwise op
                # runs once on all 4 quadrants. DMA does the partition-
                # crossing repack (rings are idle here).
                gpk = [lp.tile([128, L], dt.float16, tag=f"gp{g}",
                               name=f"gpk{g}") for g in range(4)]  # i,f,o,g
                t1 = lp.tile([128, L], dt.float16, tag="t1", name="t1")
                upk = lp.tile([128, L], dt.float16, tag="upk", name="upk")
                # h_t at col t+1 (col 0 zero); whh is row-replicated so its
                # lhsT base matches the packed rhs base per quadrant
                hpk = lp.tile([128, L + 1], dt.float16, tag="hpk", name="hpk")
                # PE base partitions are limited to 0/32/64: quadrant 3
                # (base 96) gets a DMA-unpacked copy at base 0
                hq3 = lp.tile([32, L + 1], dt.float16, tag="hq3", name="hq3")
                cbuf = lp.tile([128, L], dt.float16, tag="cbuf", name="cbuf")
                # tanh(c) lands in PSUM so the h multiply (opk SBUF + tcb
                # PSUM) dodges the equal-base-partition rule
                tcb = tp2.tile([128, L], dt.float32, tag="tcb", name="tcb")
                hfin = lp.tile([128, 256], dt.float32, tag="hfin",
                               name="hfin")
                nc.vector.memset(hpk[:, 0:1], 0.0)

                for it in range(NIT):
                    for d in range(2):
                        for s in range(SPC):
                            q = d * 2 + s
                            for c in range(NCHK):
                                xsl = xgs[:, q * L + c * CH:
                                          q * L + (c + 1) * CH]
                                ps = gp.tile([128, CH], dt.float32, tag="pg",
                                             name=f"pg{q}{c}")
                                if it == 0:
                                    for half in range(CH // 512):
                                        c0 = c * CH + half * 512
                                        pssl = ps[:, half * 512:
                                                  (half + 1) * 512]
                                        for kc in range(2):
                                            lhs = wih[:, (d * 2 + kc) * 128:
                                                      (d * 2 + kc + 1) * 128]
                                            if d == 0:
                                                rhs = seqs[s][kc][
                                                    :, PAD + c0:
                                                    PAD + c0 + 512]
                                            else:
                                                rhs = _neg_ap(
                                                    seqs[s][kc][:],
                                                    PAD + (L - 1) - c0,
                                                    -1, 512, 0, 1)
                                            nc.tensor.matmul(
                                                pssl, lhs, rhs,
                                                start=(kc == 0),
                                                stop=(kc == 1))
                                    nc.vector.tensor_copy(xsl, ps[:])
                                else:
                                    if q < 3:
                                        wsl = whh[q * 32:(q + 1) * 32,
                                                  d * 128:(d + 1) * 128]
                                    else:
                                        wsl = whh[0:32,
                                                  d * 128:(d + 1) * 128]
                                    for half in range(CH // 512):
                                        c0 = c * CH + half * 512
                                        pssl = ps[:, half * 512:
                                                  (half + 1) * 512]
                                        nc.tensor.matmul(
                                            pssl, ident[:],
                                            xgs[:, q * L + c0:
                                                q * L + c0 + 512],
                                            start=True, stop=False)
                                        if q < 3:
                                            hsl = hpk[q * 32:(q + 1) * 32,
                                                      c0:c0 + 512]
                                        else:
                                            hsl = hq3[:, c0:c0 + 512]
                                        nc.tensor.matmul(pssl, wsl, hsl,
                                                         start=False,
                                                         stop=True)
                                nc.scalar.activation(
                                    sgt[:, q * L + c * CH:
                                        q * L + (c + 1) * CH],
                                    ps[:], AF.Sigmoid)
                            # repack this quadrant's gate planes right away
                            # (overlaps the next quadrant's matmuls)
                            qc = slice(q * L, (q + 1) * L)
                            for g, eng in ((3, nc.sync), (0, nc.gpsimd),
                                           (1, nc.sync), (2, nc.gpsimd)):
                                eng.dma_start(gpk[g][q * 32:(q + 1) * 32, :],
                                              sgt[g * 32:(g + 1) * 32, qc])
                    # u = sig(i) * (2*sig(2g) - 1), c = scan: f*c + u,
                    # h = sig(o) tanh(c); scan/tanh/h chunked so they
                    # pipeline down the chain
                    for hf in range(2):
                        cs = slice(hf * 1024, (hf + 1) * 1024)
                        nc.vector.tensor_scalar(t1[:, cs], gpk[3][:, cs],
                                                2.0, 1.0, ALU.mult,
                                                ALU.subtract)
                        nc.vector.tensor_tensor(upk[:, cs], t1[:, cs],
                                                gpk[0][:, cs], ALU.mult)
                    for c in range(L // 512):
                        cs = slice(c * 512, (c + 1) * 512)
                        init = 0.0 if c == 0 else cbuf[:, c * 512 - 1:c * 512]
                        nc.vector.tensor_tensor_scan(
                            cbuf[:, cs], gpk[1][:, cs], upk[:, cs],
                            init, ALU.mult, ALU.add)
                        nc.scalar.activation(tcb[:, cs], cbuf[:, cs], AF.Tanh)
                        nc.vector.tensor_tensor(
                            hpk[:, 1 + c * 512:1 + (c + 1) * 512],
                            gpk[2][:, cs], tcb[:, cs], ALU.mult)
                        if it < NIT - 1:
                            nc.gpsimd.dma_start(
                                hq3[:, 1 + c * 512:1 + (c + 1) * 512],
                                hpk[96:128, 1 + c * 512:1 + (c + 1) * 512])

                # fwd needs h at t=8j+7 (col 8j+8 of hpk); bwd (stored
                # reversed) needs h_rev[L-1-8j] (col L-8j).
                for q in range(4):
                    src = hpk[q * 32:(q + 1) * 32, :].copy()
                    p0 = list(src.ap[0])
                    if q < 2:
                        src.ap = bass_rust.VecI64Pair([p0, [8, 256]])
                        src.offset = src.offset + 8
                    else:
                        src.ap = bass_rust.VecI64Pair([p0, [-8, 256]])
                        src.offset = src.offset + L
                    nc.vector.tensor_copy(hfin[q * 32:(q + 1) * 32, :], src)
                nc.sync.dma_start(hout_d[:], hfin[:])
                if debug:
                    for s in range(SPC):
                        for h in range(2):
                            nc.sync.dma_start(dbg_seqs_d[s, h], seqs[s][h][:])

    return nc


# ---------------------------------------------------------------- entry point

def _gather(res):
    """hout (128, 256) per core (rows = (d*2+s)*32 + hdim, cols = output
    position j) -> full (B, 256, 64) output."""
    out = np.zeros((B, 256, 64), np.float32)
    for core in range(NCORES):
        ho = res.results[core]["hout"]          # (128, 256)
        for s in range(SPC):
            bidx = core * SPC + s
            out[bidx, :, 0:32] = ho[s * 32:(s + 1) * 32, :].T
            out[bidx, :, 32:64] = ho[(2 + s) * 32:(3 + s) * 32, :].T
    return out


def kernel(**inputs):
    in_maps, meta = _host_prepare(inputs)
    nc = _build_program(meta)
    _fix_excess_waits(nc)
    res = run_bass_kernel_spmd(nc, in_maps, list(range(NCORES)))
    return _gather(res)

